# revision 1
# baseline (speedup 1.0000x reference)
"""Causal self-attention (B=4, T=2048, C=1024, H=16, D=64) on 8 trn2 cores.

Sharding: data-parallel over B (4) x tensor-parallel over head-halves (2).
Core c handles batch c//2 with heads [8*(c%2), 8*(c%2)+8). Each core emits a
partial projection output [2048, 1024]; host sums the two head-half partials
per batch and adds the (bv @ Wp + bp) correction row.

Device layout highlights:
 - all matmuls in float32r (full PE rate, ~1e-4 rel err)
 - x is pre-transposed on host, so Q^T/K^T/V all come out of natural-layout
   matmuls; S^T = K^T.T @ Q^T keeps softmax denominators computable by an
   in-matmul ones-column (V' has a 65th column of ones -> row 64 of O' = Z)
 - softmax skips max-subtraction (logits are ~N(0,1); exp cannot overflow)
 - causal masking via 0/1 mask multiply on the 4 diagonal-block patterns
 - t-chunk-outer loop interleaves QKV / attention / projection so PE stays
   busy while ACT runs the exps
"""

import os
import sys

for _p in ("/opt/trn_rl_repo", "/root/.axon_site/_ro/trn_rl_repo"):
    if os.path.isdir(_p) and _p not in sys.path:
        sys.path.insert(0, _p)

import numpy as np
from concourse import bacc, mybir, tile
from concourse.bass_utils import run_bass_kernel_spmd

N_CORES = 8
B, T, C = 4, 2048, 1024
H, D = 16, 64          # full model heads
HG = 8                 # heads per core (head-group)
CH = HG * D            # 512, per-core qkv width
NT = T // 128          # 16 s-tiles
NJ = T // 512          # 4 t-chunks
NC_ = C // 128         # 8 contraction tiles
F32 = mybir.dt.float32
F32R = mybir.dt.float32r
AF = mybir.ActivationFunctionType

_CACHE = {}


def _emit(nc, tc, aps):
    xT, wq, wk, wv, wp, bq2, bk2, mask, yout = (
        aps["xT"], aps["wq"], aps["wk"], aps["wv"], aps["wp"],
        aps["bq2"], aps["bk2"], aps["mask"], aps["y"],
    )

    pool = tc.alloc_tile_pool(name="pool", bufs=1)
    psp = tc.alloc_tile_pool(name="ps", bufs=1, space="PSUM")

    # ---- persistent tensors ----
    kt = [pool.tile([128, T], F32R, name=f"kt{m}", tag="kt", bufs=4) for m in range(4)]
    vp = [pool.tile([128, 520], F32R, name=f"vp{i}", tag="vp", bufs=NT)
          for i in range(NT)]
    # single lower-triangle mask (1{s <= t}) for the diagonal 128x128 blocks
    tri = pool.tile([128, 128], F32R, name="tri", tag="tri", bufs=1)
    bqs = pool.tile([128, 4], F32, name="bqs", tag="bias", bufs=2)
    bks = pool.tile([128, 4], F32, name="bks", tag="bias", bufs=2)
    ones = pool.tile([128, 64], F32R, name="ones", tag="ones", bufs=1)
    ones_f = pool.tile([128, 64], F32, name="ones_f", tag="ones_f", bufs=1)

    # weights: wq/wk/wv now, wp reuses the same slots once QKV is done
    W = 24  # shared slot budget for 512-wide weight tiles
    wqs = [pool.tile([128, CH], F32R, name=f"wqs{ci}", tag="w", bufs=W)
           for ci in range(NC_)]
    wks = [pool.tile([128, CH], F32R, name=f"wks{ci}", tag="w", bufs=W)
           for ci in range(NC_)]
    wvs = [pool.tile([128, CH], F32R, name=f"wvs{ci}", tag="w", bufs=W)
           for ci in range(NC_)]
    # DMA queue split (both HWDGE queues; SWDGE descriptor-gen is ~28us per
    # strided tile, so gpsimd is avoided): sync carries wq interleaved with
    # the first x chunk so QT matmuls start immediately; the scalar queue
    # carries wk/wv/bias/mask in parallel.
    xt0 = []
    for ci in range(NC_):
        nc.sync.dma_start(wqs[ci][:], wq[128 * ci:128 * ci + 128, :].bitcast(F32R))
        xt_t = pool.tile([128, 512], F32R, name=f"xt0_{ci}", tag="xt", bufs=8)
        eng = nc.sync if ci < 2 else nc.scalar
        eng.dma_start(
            xt_t[:], xT[128 * ci:128 * ci + 128, 0:512].bitcast(F32R)
        )
        xt0.append(xt_t)
    for ci in range(NC_):
        nc.sync.dma_start(wks[ci][:], wk[128 * ci:128 * ci + 128, :].bitcast(F32R))
    nc.scalar.dma_start(bqs[:], bq2[:])
    nc.scalar.dma_start(bks[:], bk2[:])
    for ci in range(NC_):
        nc.scalar.dma_start(wvs[ci][:], wv[128 * ci:128 * ci + 128, :].bitcast(F32R))
    nc.scalar.dma_start(tri[:], mask[:].bitcast(F32R))
    nc.gpsimd.memset(ones_f[:], 1.0)
    nc.vector.tensor_copy(ones[:], ones_f[:])
    for i in range(NT):
        ocol = vp[i][:, 0:520].rearrange("p (h e) -> p h e", e=65)[:, :, 64:65]
        nc.vector.tensor_copy(ocol, ones_f[:, 0:8].unsqueeze(2))

    qtc = [[None] * NJ for _ in range(4)]   # per-chunk Q^T tiles
    otc = [[None] * NJ for _ in range(4)]   # per-chunk O^T tiles
    wps = [[None, None] for _ in range(4)]  # wp [128,512] halves, loaded late

    def emit_qkv(j):
        if j == 0:
            xts = xt0
        else:
            xts = []
            for ci in range(NC_):
                xt_t = pool.tile([128, 512], F32R, name=f"xt{j}_{ci}", tag="xt",
                                 bufs=8)
                nc.sync.dma_start(
                    xt_t[:],
                    xT[128 * ci:128 * ci + 128, 512 * j:512 * j + 512].bitcast(F32R),
                )
                xts.append(xt_t)
        for wsrc, bias_t, dst, nm in ((wqs, bqs, qtc, "qt"), (wks, bks, None, "kt")):
            for m in range(4):
                ps = psp.tile([128, 512], F32, name=f"{nm}ps{j}_{m}", tag="qk", bufs=2)
                for ci in range(NC_):
                    nc.tensor.matmul(
                        ps[:], wsrc[ci][:, 128 * m:128 * m + 128], xts[ci][:],
                        start=(ci == 0), stop=(ci == NC_ - 1),
                    )
                if dst is None:
                    out_ap = kt[m][:, 512 * j:512 * j + 512]
                else:
                    t_ = pool.tile([128, 512], F32R, name=f"qt{m}_{j}", tag="qtc",
                                   bufs=8)
                    dst[m][j] = t_
                    out_ap = t_[:]
                nc.vector.tensor_scalar_add(out_ap, ps[:], bias_t[:, m:m + 1])
        for u in range(4):
            i = 4 * j + u
            ps = psp.tile([128, 512], F32, name=f"vps{i}", tag="qk", bufs=2)
            for ci in range(NC_):
                nc.tensor.matmul(
                    ps[:], xts[ci][:, 128 * u:128 * u + 128], wvs[ci][:],
                    start=(ci == 0), stop=(ci == NC_ - 1),
                )
            dst = vp[i][:, 0:520].rearrange("p (h e) -> p h e", e=65)[:, :, 0:64]
            src = ps[:].rearrange("p (h e) -> p h e", e=64)
            nc.vector.tensor_copy(dst, src)

    def emit_attn(j, heads=(1, 0, 3, 2, 5, 4, 7, 6)):
        n_i = 4 * j + 4

        def tile_layout(p):
            # pairs of s-tiles per [128,1024] PSUM slot; diagonal tiles are
            # narrowed to the causally valid t-range [128r, 512).
            # entries: (i, slot_col, valid_t0, width, diag_block_col)
            i0, i1 = 2 * p, 2 * p + 1
            r0_, r1_ = i0 - 4 * j, i1 - 4 * j
            if r1_ < 0:
                return [(i0, 0, 0, 512, None), (i1, 512, 0, 512, None)], 1024
            if r0_ == 0:
                return [(i0, 0, 0, 512, 0), (i1, 512, 128, 384, 512)], 896
            return [(i0, 0, 256, 256, 0), (i1, 256, 384, 128, 256)], 384

        # odd heads first: their normalize chain ends in a partition-shifting
        # SBUF->SBUF DMA, so keep an even (cheap-chain) head last
        for h in heads:
            mt = h // 2
            off = 64 * (h % 2)
            ops = psp.tile([65, 512], F32, name=f"ops{h}_{j}", tag="o", bufs=2)
            qsrc = qtc[mt][j][off:off + 64, :]
            for p in range(n_i // 2):
                layout, exp_hi = tile_layout(p)
                sp = psp.tile([128, 1024], F32, name=f"sp{h}_{j}_{p}", tag="sp",
                              bufs=2)
                for (i, scol, t0, w, _) in layout:
                    nc.tensor.matmul(
                        sp[:, scol:scol + w],
                        kt[mt][off:off + 64, 128 * i:128 * i + 128],
                        qsrc[:, t0:t0 + w],
                        start=True, stop=True,
                    )
                et = pool.tile([128, 1024], F32R, name=f"et{h}_{j}_{p}", tag="et",
                               bufs=3)
                nc.scalar.activation(et[:, 0:exp_hi], sp[:, 0:exp_hi], AF.Exp,
                                     scale=0.125)
                for (i, scol, t0, w, dcol) in layout:
                    if dcol is not None:
                        blk = et[:, dcol:dcol + 128]
                        nc.vector.tensor_mul(blk, blk, tri[:])
                    nc.tensor.matmul(
                        ops[:, t0:t0 + w], vp[i][:, 65 * h:65 * h + 65],
                        et[:, scol:scol + w],
                        start=(i == 0), stop=(i == n_i - 1),
                    )
            # normalize: rows 0..63 unnormalized O^T, row 64 = Z
            zr = pool.tile([65, 512], F32R, name=f"zr{h}_{j}", tag="zr", bufs=2)
            nc.vector.tensor_copy(zr[64:65, :], ops[64:65, :])
            rbp = psp.tile([64, 512], F32, name=f"rbp{h}_{j}", tag="o", bufs=2)
            nc.tensor.matmul(rbp[:], ones[64:65, :], zr[64:65, :], start=True,
                             stop=True)
            rbs = pool.tile([64, 512], F32R, name=f"rbs{h}_{j}", tag="rbs", bufs=2)
            with nc.allow_low_precision(reason="fp32r rounding of softmax denom"):
                nc.vector.reciprocal(rbs[:], rbp[:])
            if otc[mt][j] is None:
                otc[mt][j] = pool.tile([128, 512], F32R, name=f"ot{mt}_{j}",
                                       tag="otc", bufs=8)
            if h % 2 == 0:
                nc.vector.tensor_mul(otc[mt][j][0:64, :], ops[0:64, :], rbs[:])
            else:
                st = pool.tile([64, 512], F32R, name=f"st{h}_{j}", tag="st", bufs=1)
                nc.vector.tensor_mul(st[:], ops[0:64, :], rbs[:])
                nc.sync.dma_start(otc[mt][j][64:128, :], st[:])

    def emit_wp_loads():
        for m in range(4):
            for n in range(2):
                t_ = pool.tile([128, 512], F32R, name=f"wps{m}_{n}", tag="w", bufs=W)
                wps[m][n] = t_
                nc.sync.dma_start(
                    t_[:],
                    wp[128 * m:128 * m + 128, 512 * n:512 * n + 512].bitcast(F32R),
                )

    def emit_proj(j):
        for u in range(4):
            t = 4 * j + u
            for n in range(2):
                ps = psp.tile([128, 512], F32, name=f"yps{t}_{n}", tag="qk", bufs=2)
                for m in range(4):
                    nc.tensor.matmul(
                        ps[:], otc[m][j][:, 128 * u:128 * u + 128], wps[m][n][:],
                        start=(m == 0), stop=(m == 3),
                    )
                yo = pool.tile([128, 512], F32, name=f"yo{t}_{n}", tag="yo", bufs=2)
                nc.vector.tensor_copy(yo[:], ps[:])
                nc.sync.dma_start(
                    yout[128 * t:128 * t + 128, 512 * n:512 * n + 512], yo[:]
                )

    emit_qkv(0)
    emit_attn(0)
    emit_qkv(1)
    emit_qkv(2)
    emit_attn(1)
    emit_qkv(3)
    emit_wp_loads()
    emit_attn(2, heads=(1, 0, 3, 2))
    emit_attn(3, heads=(1, 0))
    emit_proj(0)
    emit_attn(2, heads=(5, 4, 7, 6))
    emit_attn(3, heads=(3, 2))
    emit_proj(1)
    emit_attn(3, heads=(5, 4, 7, 6))
    emit_proj(2)
    emit_proj(3)

    for m in range(4):
        qtc[m] = [None] * NJ
        otc[m] = [None] * NJ
    pool.release()
    psp.release()


def build(passes=1):
    key = ("nc", passes)
    if key in _CACHE:
        return _CACHE[key]
    nc = bacc.Bacc("TRN2", target_bir_lowering=False, debug=False,
                   num_devices=N_CORES)
    aps = {
        "xT": nc.dram_tensor("xT", [C, T], F32, kind="ExternalInput").ap(),
        "wq": nc.dram_tensor("wq", [C, CH], F32, kind="ExternalInput").ap(),
        "wk": nc.dram_tensor("wk", [C, CH], F32, kind="ExternalInput").ap(),
        "wv": nc.dram_tensor("wv", [C, CH], F32, kind="ExternalInput").ap(),
        "wp": nc.dram_tensor("wp", [CH, C], F32, kind="ExternalInput").ap(),
        "bq2": nc.dram_tensor("bq2", [128, 4], F32, kind="ExternalInput").ap(),
        "bk2": nc.dram_tensor("bk2", [128, 4], F32, kind="ExternalInput").ap(),
        "mask": nc.dram_tensor("mask", [128, 128], F32, kind="ExternalInput").ap(),
        "y": nc.dram_tensor("y", [T, C], F32, kind="ExternalOutput").ap(),
    }
    with tile.TileContext(nc) as tc:
        for _ in range(passes):
            _emit(nc, tc, aps)
    nc.compile()
    _CACHE[key] = nc
    return nc


def make_in_maps(x, Wq, bq, Wk, bk, Wv, bv, Wp, bp):
    # lower-triangle 0/1 mask for the diagonal 128x128 attention blocks
    s_idx = np.arange(128)[:, None]
    t_idx = np.arange(128)[None, :]
    mask = (s_idx <= t_idx).astype(np.float32)
    in_maps = []
    for c in range(N_CORES):
        b, g = c // 2, c % 2
        cols = slice(CH * g, CH * g + CH)
        in_maps.append({
            "xT": np.ascontiguousarray(x[b].T),
            "wq": np.ascontiguousarray(Wq[:, cols]),
            "wk": np.ascontiguousarray(Wk[:, cols]),
            "wv": np.ascontiguousarray(Wv[:, cols]),
            "wp": np.ascontiguousarray(Wp[cols, :]),
            "bq2": np.ascontiguousarray(bq[cols].reshape(4, 128).T),
            "bk2": np.ascontiguousarray(bk[cols].reshape(4, 128).T),
            "mask": mask,
        })
    return in_maps


def kernel(x, Wq, bq, Wk, bk, Wv, bv, Wp, bp):
    # host-side prep is pure numpy; convert in case jax arrays are passed
    x, Wq, bq, Wk, bk, Wv, bv, Wp, bp = (
        np.asarray(a, dtype=np.float32)
        for a in (x, Wq, bq, Wk, bk, Wv, bv, Wp, bp)
    )
    nc = build()
    in_maps = make_in_maps(x, Wq, bq, Wk, bk, Wv, bv, Wp, bp)
    # the axon-proxied device occasionally reports a transient unrecoverable
    # exec state that clears on a fresh attempt; retry rather than fail
    last_err = None
    for _attempt in range(3):
        try:
            res = run_bass_kernel_spmd(nc, in_maps, core_ids=list(range(N_CORES)))
            break
        except Exception as e:  # noqa: BLE001
            last_err = e
            import time as _time
            _time.sleep(5)
    else:
        raise last_err
    corr = (bv @ Wp + bp).astype(np.float32)
    out = np.empty((B, T, C), dtype=np.float32)
    for b in range(B):
        out[b] = res.results[2 * b]["y"] + res.results[2 * b + 1]["y"] + corr
    return out



# revision 3
# speedup vs baseline: 1.0429x; 1.0429x over previous
"""Causal self-attention (B=4, T=2048, C=1024, H=16, D=64) on 8 trn2 cores.

Sharding: data-parallel over B (4) x tensor-parallel over head-halves (2).
Core c handles batch c//2 with heads [8*(c%2), 8*(c%2)+8). Each core emits a
partial projection output [2048, 1024]; host sums the two head-half partials
per batch and adds the (bv @ Wp + bp) correction row.

Device layout highlights:
 - matmul cost on PE is proportional to the output free-dim size only, so the
   attention*V product is computed in [t, d] layout: per (head, t-block of
   128, s-tile) one bf16 matmul with a 65-wide output (64 V columns + a ones
   column that yields the softmax denominator Z per t-partition). That makes
   softmax normalization a per-partition reciprocal+tensor_scalar (no
   broadcast matmuls) and halves the AV stream cost vs. the [d, t] form.
 - normalized [t, d-pair] tiles are transposed back to [d-pair, t] on the PE
   (is_transpose matmul vs. a 128x128 identity, 1.5 cyc/row) to feed the
   output projection, which keeps its natural layout.
 - S^T = K^T.T @ Q^T stays in [s, t] layout (exp on ACT consumes it there and
   AV uses 128-col t-slices of it as the stationary operand). All S matmuls
   are >= 256 wide to stay at the 1 cyc/row fp32r rate.
 - softmax skips max-subtraction (logits are ~N(0,1); exp cannot overflow)
 - causal masking via 0/1 mask multiply on the 4 diagonal-block patterns
 - exp output and V are bf16 (attention weights/values; ~0.4% quantization),
   QKV projections, S logits and the output projection remain fp32r.
"""

import os
import sys

for _p in ("/opt/trn_rl_repo", "/root/.axon_site/_ro/trn_rl_repo"):
    if os.path.isdir(_p) and _p not in sys.path:
        sys.path.insert(0, _p)

import numpy as np
from concourse import bacc, masks, mybir, tile
from concourse.bass_utils import run_bass_kernel_spmd

N_CORES = 8
B, T, C = 4, 2048, 1024
H, D = 16, 64          # full model heads
HG = 8                 # heads per core (head-group)
CH = HG * D            # 512, per-core qkv width
NT = T // 128          # 16 s-tiles
NJ = T // 512          # 4 t-chunks
NC_ = C // 128         # 8 contraction tiles
F32 = mybir.dt.float32
F32R = mybir.dt.float32r
BF16 = mybir.dt.bfloat16
AF = mybir.ActivationFunctionType

ET_BUFS = 40           # [128,512] bf16 S^T tiles alive across the head pipeline

_CACHE = {}


def _emit(nc, tc, aps):
    xT, wq, wk, wv, wp, bq2, bk2, mask, yout = (
        aps["xT"], aps["wq"], aps["wk"], aps["wv"], aps["wp"],
        aps["bq2"], aps["bk2"], aps["mask"], aps["y"],
    )

    pool = tc.alloc_tile_pool(name="pool", bufs=1)
    psp = tc.alloc_tile_pool(name="ps", bufs=1, space="PSUM")

    # ---- persistent tensors ----
    kt = [pool.tile([128, T], F32R, name=f"kt{m}", tag="kt", bufs=4) for m in range(4)]
    vp = [pool.tile([128, 520], BF16, name=f"vp{i}", tag="vp", bufs=NT)
          for i in range(NT)]
    # single lower-triangle mask (1{s <= t}) for the diagonal 128x128 blocks
    tri_f = pool.tile([128, 128], F32, name="tri_f", tag="tri_f", bufs=1)
    tri = pool.tile([128, 128], BF16, name="tri", tag="tri", bufs=1)
    ident = pool.tile([128, 128], F32R, name="ident", tag="ident", bufs=1)
    bqs = pool.tile([128, 4], F32, name="bqs", tag="bias", bufs=2)
    bks = pool.tile([128, 4], F32, name="bks", tag="bias", bufs=2)
    ones_b = pool.tile([128, 8], BF16, name="ones_b", tag="ones_b", bufs=1)

    # weights: wq/wk/wv now, wp reuses the same slots once QKV is done
    W = 24  # shared slot budget for 512-wide weight tiles
    wqs = [pool.tile([128, CH], F32R, name=f"wqs{ci}", tag="w", bufs=W)
           for ci in range(NC_)]
    wks = [pool.tile([128, CH], F32R, name=f"wks{ci}", tag="w", bufs=W)
           for ci in range(NC_)]
    wvs = [pool.tile([128, CH], F32R, name=f"wvs{ci}", tag="w", bufs=W)
           for ci in range(NC_)]
    # DMA queue split (both HWDGE queues; SWDGE descriptor-gen is ~28us per
    # strided tile, so gpsimd is avoided): sync carries wq interleaved with
    # the first x chunk so QT matmuls start immediately; the scalar queue
    # carries wk/wv/bias/mask in parallel.
    xt0 = []
    for ci in range(NC_):
        nc.sync.dma_start(wqs[ci][:], wq[128 * ci:128 * ci + 128, :].bitcast(F32R))
        xt_t = pool.tile([128, 512], F32R, name=f"xt0_{ci}", tag="xt", bufs=8)
        eng = nc.sync if ci < 2 else nc.scalar
        eng.dma_start(
            xt_t[:], xT[128 * ci:128 * ci + 128, 0:512].bitcast(F32R)
        )
        xt0.append(xt_t)
    for ci in range(NC_):
        nc.sync.dma_start(wks[ci][:], wk[128 * ci:128 * ci + 128, :].bitcast(F32R))
    nc.scalar.dma_start(bqs[:], bq2[:])
    nc.scalar.dma_start(bks[:], bk2[:])
    for ci in range(NC_):
        nc.scalar.dma_start(wvs[ci][:], wv[128 * ci:128 * ci + 128, :].bitcast(F32R))
    nc.scalar.dma_start(tri_f[:], mask[:])
    nc.vector.tensor_copy(tri[:], tri_f[:])
    nc.gpsimd.memset(ones_b[:], 1.0)
    masks.make_identity(nc, ident[:])

    qtc = [[None] * NJ for _ in range(4)]   # per-chunk Q^T tiles
    otc = [[None] * NJ for _ in range(4)]   # per-chunk O^T tiles
    wps = [[None, None] for _ in range(4)]  # wp [128,512] halves, loaded late
    o_pair = [None] * 4                     # per-pair unnormalized O psum
    nrmt = [[None] * 4 for _ in range(4)]   # per-pair normalized [t, d-pair]

    def emit_qkv(j):
        if j == 0:
            xts = xt0
        else:
            xts = []
            for ci in range(NC_):
                xt_t = pool.tile([128, 512], F32R, name=f"xt{j}_{ci}", tag="xt",
                                 bufs=8)
                nc.sync.dma_start(
                    xt_t[:],
                    xT[128 * ci:128 * ci + 128, 512 * j:512 * j + 512].bitcast(F32R),
                )
                xts.append(xt_t)
        for wsrc, bias_t, dst, nm in ((wqs, bqs, qtc, "qt"), (wks, bks, None, "kt")):
            for m in range(4):
                ps = psp.tile([128, 512], F32, name=f"{nm}ps{j}_{m}", tag="qk", bufs=2)
                for ci in range(NC_):
                    nc.tensor.matmul(
                        ps[:], wsrc[ci][:, 128 * m:128 * m + 128], xts[ci][:],
                        start=(ci == 0), stop=(ci == NC_ - 1),
                    )
                if dst is None:
                    out_ap = kt[m][:, 512 * j:512 * j + 512]
                else:
                    t_ = pool.tile([128, 512], F32R, name=f"qt{m}_{j}", tag="qtc",
                                   bufs=8)
                    dst[m][j] = t_
                    out_ap = t_[:]
                nc.vector.tensor_scalar_add(out_ap, ps[:], bias_t[:, m:m + 1])
        for u in range(4):
            i = 4 * j + u
            ps = psp.tile([128, 512], F32, name=f"vps{i}", tag="qk", bufs=2)
            for ci in range(NC_):
                nc.tensor.matmul(
                    ps[:], xts[ci][:, 128 * u:128 * u + 128], wvs[ci][:],
                    start=(ci == 0), stop=(ci == NC_ - 1),
                )
            dst = vp[i][:, 0:520].rearrange("p (h e) -> p h e", e=65)[:, :, 0:64]
            src = ps[:].rearrange("p (h e) -> p h e", e=64)
            nc.vector.tensor_copy(dst, src)
            ocol = vp[i][:, 0:520].rearrange("p (h e) -> p h e", e=65)[:, :, 64:65]
            nc.vector.tensor_copy(ocol, ones_b[:].unsqueeze(2))

    # s-tile descriptor for chunk j: r = i - 4j; returns (t0, w) of the
    # computed S^T block [128 s, w t] at chunk-local t offset t0.
    def s_desc(r):
        if r <= 0:
            return 0, 512
        if r == 1:
            return 128, 384
        return 256, 256  # r in (2, 3); r==3 only needs [384,512) but a
        #                  256-wide matmul avoids the <256 fp32r 4x penalty

    av_pend = []  # cross-call head pipeline: emit AV one S-head later

    def emit_s_head(j, h):
        mt, off = h // 2, 64 * (h % 2)
        qsrc = qtc[mt][j]
        n_i = 4 * j + 4
        ets = []
        for i in range(n_i):
            r = i - 4 * j
            t0, w = s_desc(r)
            sp = psp.tile([128, 512], F32, name=f"sp{h}_{j}_{i}", tag="sp", bufs=3)
            nc.tensor.matmul(
                sp[:, 0:w], kt[mt][off:off + 64, 128 * i:128 * i + 128],
                qsrc[off:off + 64, t0:t0 + w], start=True, stop=True,
            )
            et = pool.tile([128, 512], BF16, name=f"et{h}_{j}_{i}", tag="et",
                           bufs=ET_BUFS)
            e0 = 128 if r == 3 else 0  # r==3: cols [0,128) are acausal garbage
            nc.scalar.activation(et[:, e0:w], sp[:, e0:w], AF.Exp, scale=0.125)
            if r >= 0:
                dc = 128 * r - t0  # in-tile col of the diagonal 128x128 block
                nc.vector.tensor_mul(et[:, dc:dc + 128], et[:, dc:dc + 128], tri[:])
            ets.append((et, t0))
        av_pend.append((j, h, ets))

    def emit_av_head():
        j, h, ets = av_pend.pop(0)
        mt, off = h // 2, 64 * (h % 2)
        op = psp.tile([128, 260], F32, name=f"o{h}_{j}", tag="o", bufs=2)
        for u in range(4):
            oc = 65 * u
            lo = 4 * j + u + 1
            for i in range(lo):
                et, t0 = ets[i]
                col = 128 * u - t0
                nc.tensor.matmul(
                    op[:, oc:oc + 65], et[:, col:col + 128],
                    vp[i][:, 65 * h:65 * h + 65],
                    start=(i == 0), stop=(i == lo - 1),
                )
        # normalize in [t, d] layout: Z is column 64 -> per-partition scalar
        for u in range(4):
            oc = 65 * u
            rb = pool.tile([128, 1], F32, name=f"rb{h}_{j}_{u}", tag="rb", bufs=8)
            nc.vector.reciprocal(rb[:], op[:, oc + 64:oc + 65])
            if h % 2 == 0:
                nrmt[mt][u] = pool.tile([128, 128], F32R, name=f"nrm{mt}_{j}_{u}",
                                        tag="nrm", bufs=8)
            nc.vector.tensor_scalar_mul(
                nrmt[mt][u][:, off:off + 64], op[:, oc:oc + 64], rb[:]
            )
        if h % 2 == 1:
            # pair complete: transpose [t, d-pair] -> [d-pair, t] for the proj
            tp = psp.tile([128, 512], F32R, name=f"tp{mt}_{j}", tag="tp", bufs=1)
            if otc[mt][j] is None:
                otc[mt][j] = pool.tile([128, 512], F32R, name=f"ot{mt}_{j}",
                                       tag="otc", bufs=8)
            for u in range(4):
                nc.tensor.matmul(
                    tp[:, 128 * u:128 * u + 128], nrmt[mt][u][:], ident[:],
                    is_transpose=True,
                )
                nc.vector.tensor_copy(
                    otc[mt][j][:, 128 * u:128 * u + 128], tp[:, 128 * u:128 * u + 128]
                )

    def emit_attn(j, heads=(0, 1, 2, 3, 4, 5, 6, 7), flush=False):
        for h in heads:
            emit_s_head(j, h)
            if len(av_pend) > 1:
                emit_av_head()
        if flush:
            while av_pend:
                emit_av_head()

    def emit_wp_loads():
        for m in range(4):
            for n in range(2):
                t_ = pool.tile([128, 512], F32R, name=f"wps{m}_{n}", tag="w", bufs=W)
                wps[m][n] = t_
                nc.sync.dma_start(
                    t_[:],
                    wp[128 * m:128 * m + 128, 512 * n:512 * n + 512].bitcast(F32R),
                )

    def emit_proj(j):
        for u in range(4):
            t = 4 * j + u
            for n in range(2):
                ps = psp.tile([128, 512], F32, name=f"yps{t}_{n}", tag="qk", bufs=2)
                for m in range(4):
                    nc.tensor.matmul(
                        ps[:], otc[m][j][:, 128 * u:128 * u + 128], wps[m][n][:],
                        start=(m == 0), stop=(m == 3),
                    )
                yo = pool.tile([128, 512], F32, name=f"yo{t}_{n}", tag="yo", bufs=2)
                nc.vector.tensor_copy(yo[:], ps[:])
                nc.sync.dma_start(
                    yout[128 * t:128 * t + 128, 512 * n:512 * n + 512], yo[:]
                )

    emit_qkv(0)
    emit_attn(0, flush=True)
    emit_qkv(1)
    emit_qkv(2)
    emit_attn(1, flush=True)
    emit_qkv(3)
    emit_wp_loads()
    emit_attn(2, heads=(0, 1, 2, 3))
    emit_attn(3, heads=(0, 1))
    emit_proj(0)
    emit_attn(2, heads=(4, 5, 6, 7))
    emit_attn(3, heads=(2, 3))
    emit_proj(1)
    emit_attn(3, heads=(4, 5, 6, 7), flush=True)
    emit_proj(2)
    emit_proj(3)

    for m in range(4):
        qtc[m] = [None] * NJ
        otc[m] = [None] * NJ
    pool.release()
    psp.release()


def build(passes=1):
    key = ("nc", passes)
    if key in _CACHE:
        return _CACHE[key]
    nc = bacc.Bacc("TRN2", target_bir_lowering=False, debug=False,
                   num_devices=N_CORES)
    aps = {
        "xT": nc.dram_tensor("xT", [C, T], F32, kind="ExternalInput").ap(),
        "wq": nc.dram_tensor("wq", [C, CH], F32, kind="ExternalInput").ap(),
        "wk": nc.dram_tensor("wk", [C, CH], F32, kind="ExternalInput").ap(),
        "wv": nc.dram_tensor("wv", [C, CH], F32, kind="ExternalInput").ap(),
        "wp": nc.dram_tensor("wp", [CH, C], F32, kind="ExternalInput").ap(),
        "bq2": nc.dram_tensor("bq2", [128, 4], F32, kind="ExternalInput").ap(),
        "bk2": nc.dram_tensor("bk2", [128, 4], F32, kind="ExternalInput").ap(),
        "mask": nc.dram_tensor("mask", [128, 128], F32, kind="ExternalInput").ap(),
        "y": nc.dram_tensor("y", [T, C], F32, kind="ExternalOutput").ap(),
    }
    with tile.TileContext(nc) as tc:
        for _ in range(passes):
            _emit(nc, tc, aps)
    nc.compile()
    _CACHE[key] = nc
    return nc


def make_in_maps(x, Wq, bq, Wk, bk, Wv, bv, Wp, bp):
    # lower-triangle 0/1 mask for the diagonal 128x128 attention blocks
    s_idx = np.arange(128)[:, None]
    t_idx = np.arange(128)[None, :]
    mask = (s_idx <= t_idx).astype(np.float32)
    in_maps = []
    for c in range(N_CORES):
        b, g = c // 2, c % 2
        cols = slice(CH * g, CH * g + CH)
        in_maps.append({
            "xT": np.ascontiguousarray(x[b].T),
            "wq": np.ascontiguousarray(Wq[:, cols]),
            "wk": np.ascontiguousarray(Wk[:, cols]),
            "wv": np.ascontiguousarray(Wv[:, cols]),
            "wp": np.ascontiguousarray(Wp[cols, :]),
            "bq2": np.ascontiguousarray(bq[cols].reshape(4, 128).T),
            "bk2": np.ascontiguousarray(bk[cols].reshape(4, 128).T),
            "mask": mask,
        })
    return in_maps


def kernel(x, Wq, bq, Wk, bk, Wv, bv, Wp, bp):
    # host-side prep is pure numpy; convert in case jax arrays are passed
    x, Wq, bq, Wk, bk, Wv, bv, Wp, bp = (
        np.asarray(a, dtype=np.float32)
        for a in (x, Wq, bq, Wk, bk, Wv, bv, Wp, bp)
    )
    nc = build()
    in_maps = make_in_maps(x, Wq, bq, Wk, bk, Wv, bv, Wp, bp)
    # the axon-proxied device occasionally reports a transient unrecoverable
    # exec state that clears on a fresh attempt; retry rather than fail
    last_err = None
    for _attempt in range(3):
        try:
            res = run_bass_kernel_spmd(nc, in_maps, core_ids=list(range(N_CORES)))
            break
        except Exception as e:  # noqa: BLE001
            last_err = e
            import time as _time
            _time.sleep(5)
    else:
        raise last_err
    corr = (bv @ Wp + bp).astype(np.float32)
    out = np.empty((B, T, C), dtype=np.float32)
    for b in range(B):
        out[b] = res.results[2 * b]["y"] + res.results[2 * b + 1]["y"] + corr
    return out


# revision 5
# speedup vs baseline: 1.0923x; 1.0474x over previous
"""Causal self-attention (B=4, T=2048, C=1024, H=16, D=64) on 8 trn2 cores.

Sharding: data-parallel over B (4) x tensor-parallel over head-halves (2).
Core c handles batch c//2 with heads [8*(c%2), 8*(c%2)+8). Each core emits a
partial projection output [2048, 1024]; host sums the two head-half partials
per batch and adds the (bv @ Wp + bp) correction row.

Device layout highlights:
 - matmul cost on PE is proportional to the output free-dim size only, so the
   attention*V product is computed in [t, d] layout: per (head, t-block of
   128, s-tile) one bf16 matmul with a 65-wide output (64 V columns + a ones
   column that yields the softmax denominator Z per t-partition). That makes
   softmax normalization a per-partition reciprocal+tensor_scalar (no
   broadcast matmuls) and halves the AV stream cost vs. the [d, t] form.
 - normalized [t, d-pair] tiles are transposed back to [d-pair, t] on the PE
   (is_transpose matmul vs. a 128x128 identity, 1.5 cyc/row) to feed the
   output projection, which keeps its natural layout.
 - S^T = K^T.T @ Q^T stays in [s, t] layout in [128,1024] psum pair slots
   (one exp instruction per slot amortizes ACT's fixed per-instruction
   overhead); all S matmuls are >= 256 wide for the 1 cyc/row fp32r rate.
 - the attention phase is ACT(exp)-bound, so emission interleaves S pair
   units with the previous head's AV chains and with projection / QKV
   chains at fine granularity: the in-order PE stream always has non-S work
   between S units that would otherwise stall on the exp pipeline.
 - softmax skips max-subtraction (logits are ~N(0,1); exp cannot overflow)
 - causal masking via 0/1 mask multiply on the 4 diagonal-block patterns
 - exp output and V are bf16 (attention weights/values; ~0.4% quantization),
   QKV projections, S logits and the output projection remain fp32r.
"""

import os
import sys

for _p in ("/opt/trn_rl_repo", "/root/.axon_site/_ro/trn_rl_repo"):
    if os.path.isdir(_p) and _p not in sys.path:
        sys.path.insert(0, _p)

import numpy as np
from concourse import bacc, masks, mybir, tile
from concourse.bass_utils import run_bass_kernel_spmd

N_CORES = 8
B, T, C = 4, 2048, 1024
H, D = 16, 64          # full model heads
HG = 8                 # heads per core (head-group)
CH = HG * D            # 512, per-core qkv width
NT = T // 128          # 16 s-tiles
NJ = T // 512          # 4 t-chunks
NC_ = C // 128         # 8 contraction tiles
F32 = mybir.dt.float32
F32R = mybir.dt.float32r
BF16 = mybir.dt.bfloat16
AF = mybir.ActivationFunctionType

ET_BUFS = 17           # [128,1024] bf16 S^T pair tiles across the head pipeline

_CACHE = {}


def _emit(nc, tc, aps):
    xT, wq, wk, wv, wp, bq2, bk2, mask, yout = (
        aps["xT"], aps["wq"], aps["wk"], aps["wv"], aps["wp"],
        aps["bq2"], aps["bk2"], aps["mask"], aps["y"],
    )

    pool = tc.alloc_tile_pool(name="pool", bufs=1)
    psp = tc.alloc_tile_pool(name="ps", bufs=1, space="PSUM")

    # ---- persistent tensors ----
    kt = [pool.tile([128, T], F32R, name=f"kt{m}", tag="kt", bufs=4) for m in range(4)]
    vp = [pool.tile([128, 520], BF16, name=f"vp{i}", tag="vp", bufs=NT)
          for i in range(NT)]
    # single lower-triangle mask (1{s <= t}) for the diagonal 128x128 blocks
    tri_f = pool.tile([128, 128], F32, name="tri_f", tag="tri_f", bufs=1)
    tri = pool.tile([128, 128], BF16, name="tri", tag="tri", bufs=1)
    ident = pool.tile([128, 128], F32R, name="ident", tag="ident", bufs=1)
    bqs = pool.tile([128, 4], F32, name="bqs", tag="bias", bufs=2)
    bks = pool.tile([128, 4], F32, name="bks", tag="bias", bufs=2)
    ones_b = pool.tile([128, 8], BF16, name="ones_b", tag="ones_b", bufs=1)

    # weights: wq/wk/wv now, wp reuses the same slots once QKV is done
    W = 24  # shared slot budget for 512-wide weight tiles
    wqs = [pool.tile([128, CH], F32R, name=f"wqs{ci}", tag="w", bufs=W)
           for ci in range(NC_)]
    wks = [pool.tile([128, CH], F32R, name=f"wks{ci}", tag="w", bufs=W)
           for ci in range(NC_)]
    wvs = [pool.tile([128, CH], F32R, name=f"wvs{ci}", tag="w", bufs=W)
           for ci in range(NC_)]
    # DMA queue split (both HWDGE queues; SWDGE descriptor-gen is ~28us per
    # strided tile, so gpsimd is avoided): sync carries wq interleaved with
    # the first x chunk so QT matmuls start immediately; the scalar queue
    # carries wk/wv/bias/mask in parallel.
    xts_all = [[None] * NC_ for _ in range(NJ)]
    for ci in range(NC_):
        nc.sync.dma_start(wqs[ci][:], wq[128 * ci:128 * ci + 128, :].bitcast(F32R))
        xt_t = pool.tile([128, 512], F32R, name=f"xt0_{ci}", tag="xt", bufs=8)
        eng = nc.sync if ci < 2 else nc.scalar
        eng.dma_start(
            xt_t[:], xT[128 * ci:128 * ci + 128, 0:512].bitcast(F32R)
        )
        xts_all[0][ci] = xt_t
    for ci in range(NC_):
        nc.sync.dma_start(wks[ci][:], wk[128 * ci:128 * ci + 128, :].bitcast(F32R))
    nc.scalar.dma_start(bqs[:], bq2[:])
    nc.scalar.dma_start(bks[:], bk2[:])
    for ci in range(NC_):
        nc.scalar.dma_start(wvs[ci][:], wv[128 * ci:128 * ci + 128, :].bitcast(F32R))
    nc.scalar.dma_start(tri_f[:], mask[:])
    nc.vector.tensor_copy(tri[:], tri_f[:])
    nc.gpsimd.memset(ones_b[:], 1.0)
    masks.make_identity(nc, ident[:])

    qtc = [[None] * NJ for _ in range(4)]   # per-chunk Q^T tiles
    otc = [[None] * NJ for _ in range(4)]   # per-chunk O^T tiles
    wps = [[None, None] for _ in range(4)]  # wp [128,512] halves, loaded late
    nrmt = [[None] * 4 for _ in range(4)]   # per-pair normalized [t, d-pair]

    def emit_qkv_dma(j):
        if j == 0:
            return
        for ci in range(NC_):
            xt_t = pool.tile([128, 512], F32R, name=f"xt{j}_{ci}", tag="xt",
                             bufs=8)
            nc.sync.dma_start(
                xt_t[:],
                xT[128 * ci:128 * ci + 128, 512 * j:512 * j + 512].bitcast(F32R),
            )
            xts_all[j][ci] = xt_t

    def qkv_chain(j, kind, m):
        # one [128,512] psum accumulation chain of the q/k/v projections
        xts = xts_all[j]
        if kind == "v":
            i = 4 * j + m
            ps = psp.tile([128, 512], F32, name=f"vps{i}", tag="qk", bufs=2)
            for ci in range(NC_):
                nc.tensor.matmul(
                    ps[:], xts[ci][:, 128 * m:128 * m + 128], wvs[ci][:],
                    start=(ci == 0), stop=(ci == NC_ - 1),
                )
            dst = vp[i][:, 0:520].rearrange("p (h e) -> p h e", e=65)[:, :, 0:64]
            src = ps[:].rearrange("p (h e) -> p h e", e=64)
            nc.vector.tensor_copy(dst, src)
            ocol = vp[i][:, 0:520].rearrange("p (h e) -> p h e", e=65)[:, :, 64:65]
            nc.vector.tensor_copy(ocol, ones_b[:].unsqueeze(2))
            return
        wsrc, bias_t = (wqs, bqs) if kind == "q" else (wks, bks)
        ps = psp.tile([128, 512], F32, name=f"{kind}ps{j}_{m}", tag="qk", bufs=2)
        for ci in range(NC_):
            nc.tensor.matmul(
                ps[:], wsrc[ci][:, 128 * m:128 * m + 128], xts[ci][:],
                start=(ci == 0), stop=(ci == NC_ - 1),
            )
        if kind == "k":
            out_ap = kt[m][:, 512 * j:512 * j + 512]
        else:
            t_ = pool.tile([128, 512], F32R, name=f"qt{m}_{j}", tag="qtc", bufs=8)
            qtc[m][j] = t_
            out_ap = t_[:]
        nc.vector.tensor_scalar_add(out_ap, ps[:], bias_t[:, m:m + 1])

    def qkv_units(j):
        # q first (unblocks attention), then k, then v
        return ([lambda j=j, m=m: qkv_chain(j, "q", m) for m in range(4)]
                + [lambda j=j, m=m: qkv_chain(j, "k", m) for m in range(4)]
                + [lambda j=j, m=m: qkv_chain(j, "v", m) for m in range(4)])

    def emit_qkv(j):
        emit_qkv_dma(j)
        for f in qkv_units(j):
            f()

    # ---- attention ----
    # S^T pair-slot descriptors for chunk j: list of slots, each a list of
    # (i, col0, t0, w, mask_col). AV consumption: col = col0 + 128*u - t0.
    def s_slots(j):
        out = []
        for p in range(2 * j):  # full pairs
            out.append([(2 * p, 0, 0, 512, None), (2 * p + 1, 512, 0, 512, None)])
        # diagonal pair A: tiles 4j (full width) and 4j+1 (t >= 128)
        out.append([(4 * j, 0, 0, 512, 0), (4 * j + 1, 512, 128, 384, 512)])
        # diagonal pair B: tiles 4j+2 and 4j+3, both 256 wide at t0=256
        # (tile 4j+3 only needs t in [384,512) but 256-wide matmuls avoid the
        # <256 fp32r 4x penalty; its cols [256,384) are acausal garbage that
        # the exp covers harmlessly and AV never reads)
        out.append([(4 * j + 2, 0, 256, 256, 0), (4 * j + 3, 256, 256, 256, 384)])
        return out

    av_pend = []  # cross-call head pipeline: AV trails S by one head

    def s_units(j, h):
        mt, off = h // 2, 64 * (h % 2)
        ets = {}  # s-tile i -> (et tile, col0, t0)

        def emit_slot(slot):
            qsrc = qtc[mt][j]
            wtot = max(c0 + w for (_, c0, _, w, _) in slot)
            sp = psp.tile([128, 1024], F32, name=f"sp{h}_{j}", tag="sp", bufs=2)
            et = pool.tile([128, 1024], BF16, name=f"et{h}_{j}", tag="et",
                           bufs=ET_BUFS)
            for (i, c0, t0, w, _) in slot:
                nc.tensor.matmul(
                    sp[:, c0:c0 + w], kt[mt][off:off + 64, 128 * i:128 * i + 128],
                    qsrc[off:off + 64, t0:t0 + w], start=True, stop=True,
                )
            nc.scalar.activation(et[:, 0:wtot], sp[:, 0:wtot], AF.Exp, scale=0.125)
            for (i, c0, t0, w, mcol) in slot:
                if mcol is not None:
                    nc.vector.tensor_mul(et[:, mcol:mcol + 128],
                                         et[:, mcol:mcol + 128], tri[:])
                ets[i] = (et, c0, t0)

        units = [lambda slot=slot: emit_slot(slot) for slot in s_slots(j)]
        return units, ets

    def av_units(j, h, ets):
        mt, off = h // 2, 64 * (h % 2)
        op = psp.tile([128, 260], F32, name=f"o{h}_{j}", tag="o", bufs=2)

        def chain(u):
            oc = 65 * u
            lo = 4 * j + u + 1
            for i in range(lo):
                et, c0, t0 = ets[i]
                col = c0 + 128 * u - t0
                nc.tensor.matmul(
                    op[:, oc:oc + 65], et[:, col:col + 128],
                    vp[i][:, 65 * h:65 * h + 65],
                    start=(i == 0), stop=(i == lo - 1),
                )
            # normalize in [t, d]: Z is column 64 -> per-partition scalar
            rb = pool.tile([128, 1], F32, name=f"rb{h}_{j}_{u}", tag="rb", bufs=8)
            nc.vector.reciprocal(rb[:], op[:, oc + 64:oc + 65])
            if h % 2 == 0:
                nrmt[mt][u] = pool.tile([128, 128], F32R, name=f"nrm{mt}_{j}_{u}",
                                        tag="nrm", bufs=8)
            nc.vector.tensor_scalar_mul(
                nrmt[mt][u][:, off:off + 64], op[:, oc:oc + 64], rb[:]
            )

        def tp_unit():
            # pair complete: transpose [t, d-pair] -> [d-pair, t] for the proj
            tp = psp.tile([128, 512], F32R, name=f"tp{mt}_{j}", tag="qk", bufs=2)
            ot = pool.tile([128, 512], F32R, name=f"ot{mt}_{j}", tag="otc", bufs=16)
            otc[mt][j] = ot
            for u in range(4):
                nc.tensor.matmul(
                    tp[:, 128 * u:128 * u + 128], nrmt[mt][u][:], ident[:],
                    is_transpose=True,
                )
                nc.vector.tensor_copy(
                    ot[:, 128 * u:128 * u + 128], tp[:, 128 * u:128 * u + 128]
                )

        units = [lambda u=u: chain(u) for u in range(4)]
        if h % 2 == 1:
            units.append(tp_unit)
        return units

    def attn_heads(j, heads, extras=()):
        """Emit S for each head, interleaving the previous head's AV chains
        and any extra PE work units (proj/qkv chains) between S pair-slots."""
        extras = list(extras)
        for h in heads:
            su, ets = s_units(j, h)
            au = av_units(*av_pend.pop(0)) if av_pend else []
            seq = [su[0]]
            si, ai = 1, 0
            while si < len(su) or ai < len(au):
                if ai < len(au):
                    seq.append(au[ai])
                    ai += 1
                if si < len(su):
                    seq.append(su[si])
                    si += 1
            for f in seq:
                f()
            if extras:
                extras.pop(0)()
            av_pend.append((j, h, ets))
        for f in extras:
            f()

    def flush_av():
        while av_pend:
            for f in av_units(*av_pend.pop(0)):
                f()

    def emit_wp_loads():
        for m in range(4):
            for n in range(2):
                t_ = pool.tile([128, 512], F32R, name=f"wps{m}_{n}", tag="w", bufs=W)
                wps[m][n] = t_
                nc.sync.dma_start(
                    t_[:],
                    wp[128 * m:128 * m + 128, 512 * n:512 * n + 512].bitcast(F32R),
                )

    def proj_chain(j, u, n):
        t = 4 * j + u
        ps = psp.tile([128, 512], F32, name=f"yps{t}_{n}", tag="qk", bufs=2)
        for m in range(4):
            nc.tensor.matmul(
                ps[:], otc[m][j][:, 128 * u:128 * u + 128], wps[m][n][:],
                start=(m == 0), stop=(m == 3),
            )
        yo = pool.tile([128, 512], F32, name=f"yo{t}_{n}", tag="yo", bufs=3)
        nc.vector.tensor_copy(yo[:], ps[:])
        nc.sync.dma_start(
            yout[128 * t:128 * t + 128, 512 * n:512 * n + 512], yo[:]
        )

    def proj_units(j):
        return [lambda j=j, u=u, n=n: proj_chain(j, u, n)
                for u in range(4) for n in range(2)]

    def group(units, sizes):
        out, k = [], 0
        for s in sizes:
            chunk = units[k:k + s]
            out.append(lambda chunk=chunk: [f() for f in chunk])
            k += s
        assert k == len(units)
        return out

    # ---- macro schedule ----
    emit_qkv(0)
    attn_heads(0, range(8))
    emit_qkv(1)
    emit_qkv(2)
    emit_qkv_dma(3)
    attn_heads(1, range(8), extras=group(qkv_units(3), (2, 2, 2, 2, 1, 1, 1, 1)))
    emit_wp_loads()
    attn_heads(2, range(8), extras=group(proj_units(0), (1,) * 8))
    attn_heads(3, range(6),
               extras=group(proj_units(1) + proj_units(2)[:4],
                            (2, 2, 2, 2, 2, 2)))
    attn_heads(3, range(6, 8), extras=group(proj_units(2)[4:], (2, 2)))
    flush_av()
    for f in proj_units(3):
        f()

    for m in range(4):
        qtc[m] = [None] * NJ
        otc[m] = [None] * NJ
    pool.release()
    psp.release()


def build(passes=1):
    key = ("nc", passes)
    if key in _CACHE:
        return _CACHE[key]
    nc = bacc.Bacc("TRN2", target_bir_lowering=False, debug=False,
                   num_devices=N_CORES)
    aps = {
        "xT": nc.dram_tensor("xT", [C, T], F32, kind="ExternalInput").ap(),
        "wq": nc.dram_tensor("wq", [C, CH], F32, kind="ExternalInput").ap(),
        "wk": nc.dram_tensor("wk", [C, CH], F32, kind="ExternalInput").ap(),
        "wv": nc.dram_tensor("wv", [C, CH], F32, kind="ExternalInput").ap(),
        "wp": nc.dram_tensor("wp", [CH, C], F32, kind="ExternalInput").ap(),
        "bq2": nc.dram_tensor("bq2", [128, 4], F32, kind="ExternalInput").ap(),
        "bk2": nc.dram_tensor("bk2", [128, 4], F32, kind="ExternalInput").ap(),
        "mask": nc.dram_tensor("mask", [128, 128], F32, kind="ExternalInput").ap(),
        "y": nc.dram_tensor("y", [T, C], F32, kind="ExternalOutput").ap(),
    }
    with tile.TileContext(nc) as tc:
        for _ in range(passes):
            _emit(nc, tc, aps)
    nc.compile()
    _CACHE[key] = nc
    return nc


def make_in_maps(x, Wq, bq, Wk, bk, Wv, bv, Wp, bp):
    # lower-triangle 0/1 mask for the diagonal 128x128 attention blocks
    s_idx = np.arange(128)[:, None]
    t_idx = np.arange(128)[None, :]
    mask = (s_idx <= t_idx).astype(np.float32)
    in_maps = []
    for c in range(N_CORES):
        b, g = c // 2, c % 2
        cols = slice(CH * g, CH * g + CH)
        in_maps.append({
            "xT": np.ascontiguousarray(x[b].T),
            "wq": np.ascontiguousarray(Wq[:, cols]),
            "wk": np.ascontiguousarray(Wk[:, cols]),
            "wv": np.ascontiguousarray(Wv[:, cols]),
            "wp": np.ascontiguousarray(Wp[cols, :]),
            "bq2": np.ascontiguousarray(bq[cols].reshape(4, 128).T),
            "bk2": np.ascontiguousarray(bk[cols].reshape(4, 128).T),
            "mask": mask,
        })
    return in_maps


def kernel(x, Wq, bq, Wk, bk, Wv, bv, Wp, bp):
    # host-side prep is pure numpy; convert in case jax arrays are passed
    x, Wq, bq, Wk, bk, Wv, bv, Wp, bp = (
        np.asarray(a, dtype=np.float32)
        for a in (x, Wq, bq, Wk, bk, Wv, bv, Wp, bp)
    )
    nc = build()
    in_maps = make_in_maps(x, Wq, bq, Wk, bk, Wv, bv, Wp, bp)
    # the axon-proxied device occasionally reports a transient unrecoverable
    # exec state that clears on a fresh attempt; retry rather than fail
    last_err = None
    for _attempt in range(3):
        try:
            res = run_bass_kernel_spmd(nc, in_maps, core_ids=list(range(N_CORES)))
            break
        except Exception as e:  # noqa: BLE001
            last_err = e
            import time as _time
            _time.sleep(5)
    else:
        raise last_err
    corr = (bv @ Wp + bp).astype(np.float32)
    out = np.empty((B, T, C), dtype=np.float32)
    for b in range(B):
        out[b] = res.results[2 * b]["y"] + res.results[2 * b + 1]["y"] + corr
    return out


# revision 7
# speedup vs baseline: 1.1177x; 1.0232x over previous
"""Causal self-attention (B=4, T=2048, C=1024, H=16, D=64) on 8 trn2 cores.

Sharding: data-parallel over B (4) x tensor-parallel over head-halves (2).
Core c handles batch c//2 with heads [8*(c%2), 8*(c%2)+8). Each core emits a
partial projection output [2048, 1024] (bf16); host sums the two head-half
partials per batch and adds the (bv @ Wp + bp) correction row.

Device layout highlights:
 - matmul cost on PE is proportional to the output free-dim size only, so the
   attention*V product is computed in [t, d] layout: per (head, t-block of
   128, s-tile) one bf16 matmul with a 65-wide output (64 V columns + a ones
   column that yields the softmax denominator Z per t-partition). That makes
   softmax normalization a per-partition reciprocal+tensor_scalar (no
   broadcast matmuls) and halves the AV stream cost vs. the [d, t] form.
 - normalized [t, d-pair] tiles are transposed back to [d-pair, t] on the PE
   (is_transpose matmul vs. a 128x128 identity, 1.5 cyc/row) to feed the
   output projection; the transpose emission is delayed by one head so the
   DVE normalize chain never stalls the PE.
 - S^T = K^T.T @ Q^T stays in [s, t] layout in [128,1024] psum pair slots
   (one exp instruction per slot amortizes ACT's fixed per-instruction
   overhead); all S matmuls are >= 256 wide for the 1 cyc/row fp32r rate.
 - the attention phase is ACT(exp)-bound, so emission interleaves S pair
   units with the previous head's AV chains and with projection / QKV
   chains at fine granularity; attention chunks run in order 1,2,3,0 so the
   pipeline drains on the cheapest chunk's exps.
 - x and the QKV weights stream in as bf16 (halves the DMA startup), the S
   logit accumulation and output projection stay fp32r; y streams out bf16.
 - softmax skips max-subtraction (logits are ~N(0,1); exp cannot overflow)
 - causal masking via 0/1 mask multiply on the 4 diagonal-block patterns
"""

import os
import sys

for _p in ("/opt/trn_rl_repo", "/root/.axon_site/_ro/trn_rl_repo"):
    if os.path.isdir(_p) and _p not in sys.path:
        sys.path.insert(0, _p)

import numpy as np
from concourse import bacc, masks, mybir, tile
from concourse.bass_utils import run_bass_kernel_spmd

N_CORES = 8
B, T, C = 4, 2048, 1024
H, D = 16, 64          # full model heads
HG = 8                 # heads per core (head-group)
CH = HG * D            # 512, per-core qkv width
NT = T // 128          # 16 s-tiles
NJ = T // 512          # 4 t-chunks
NC_ = C // 128         # 8 contraction tiles
F32 = mybir.dt.float32
F32R = mybir.dt.float32r
BF16 = mybir.dt.bfloat16
U16 = mybir.dt.uint16
AF = mybir.ActivationFunctionType

ET_BUFS = 18           # [128,1024] bf16 S^T pair tiles across the head pipeline

_CACHE = {}


def _emit(nc, tc, aps):
    xT, wq, wk, wv, wp, bq2, bk2, mask, yout = (
        aps["xT"], aps["wq"], aps["wk"], aps["wv"], aps["wp"],
        aps["bq2"], aps["bk2"], aps["mask"], aps["y"],
    )

    pool = tc.alloc_tile_pool(name="pool", bufs=1)
    psp = tc.alloc_tile_pool(name="ps", bufs=1, space="PSUM")

    # ---- persistent tensors ----
    kt = [pool.tile([128, T], F32R, name=f"kt{m}", tag="kt", bufs=4) for m in range(4)]
    vp = [pool.tile([128, 520], BF16, name=f"vp{i}", tag="vp", bufs=NT)
          for i in range(NT)]
    # single lower-triangle mask (1{s <= t}) for the diagonal 128x128 blocks
    tri_f = pool.tile([128, 128], F32, name="tri_f", tag="tri_f", bufs=1)
    tri = pool.tile([128, 128], BF16, name="tri", tag="tri", bufs=1)
    ident = pool.tile([128, 128], F32R, name="ident", tag="ident", bufs=1)
    bqs = pool.tile([128, 4], F32, name="bqs", tag="bias", bufs=2)
    bks = pool.tile([128, 4], F32, name="bks", tag="bias", bufs=2)
    ones_b = pool.tile([128, 8], BF16, name="ones_b", tag="ones_b", bufs=1)

    wqs = [pool.tile([128, CH], BF16, name=f"wqs{ci}", tag="wqkv", bufs=24)
           for ci in range(NC_)]
    wks = [pool.tile([128, CH], BF16, name=f"wks{ci}", tag="wqkv", bufs=24)
           for ci in range(NC_)]
    wvs = [pool.tile([128, CH], BF16, name=f"wvs{ci}", tag="wqkv", bufs=24)
           for ci in range(NC_)]
    # DMA queue split (both HWDGE queues; SWDGE descriptor-gen is ~28us per
    # strided tile, so gpsimd is avoided): sync carries wq then wk so the Q
    # then K matmuls start immediately; the scalar queue carries the first x
    # chunk, biases, wv and the mask in parallel.
    xts_all = [[None] * NC_ for _ in range(NJ)]
    for ci in range(NC_):
        nc.sync.dma_start(wqs[ci][:], wq[128 * ci:128 * ci + 128, :].bitcast(BF16))
        xt_t = pool.tile([128, 512], BF16, name=f"xt0_{ci}", tag="xt", bufs=8)
        nc.scalar.dma_start(
            xt_t[:], xT[128 * ci:128 * ci + 128, 0:512].bitcast(BF16)
        )
        xts_all[0][ci] = xt_t
    for ci in range(NC_):
        nc.sync.dma_start(wks[ci][:], wk[128 * ci:128 * ci + 128, :].bitcast(BF16))
    nc.scalar.dma_start(bqs[:], bq2[:])
    nc.scalar.dma_start(bks[:], bk2[:])
    for ci in range(NC_):
        nc.scalar.dma_start(wvs[ci][:], wv[128 * ci:128 * ci + 128, :].bitcast(BF16))
    nc.scalar.dma_start(tri_f[:], mask[:])
    nc.vector.tensor_copy(tri[:], tri_f[:])
    nc.gpsimd.memset(ones_b[:], 1.0)
    masks.make_identity(nc, ident[:])

    qtc = [[None] * NJ for _ in range(4)]   # per-chunk Q^T tiles
    otc = [[None] * NJ for _ in range(4)]   # per-chunk O^T tiles
    wps = [[None, None] for _ in range(4)]  # wp [128,512] halves, loaded late
    nrmt = [[None] * 4 for _ in range(4)]   # per-pair normalized [t, d-pair]

    def emit_qkv_dma(j):
        if j == 0:
            return
        for ci in range(NC_):
            xt_t = pool.tile([128, 512], BF16, name=f"xt{j}_{ci}", tag="xt",
                             bufs=8)
            nc.sync.dma_start(
                xt_t[:],
                xT[128 * ci:128 * ci + 128, 512 * j:512 * j + 512].bitcast(BF16),
            )
            xts_all[j][ci] = xt_t

    def qkv_chain(j, kind, m):
        # one [128,512] psum accumulation chain of the q/k/v projections
        xts = xts_all[j]
        if kind == "v":
            i = 4 * j + m
            ps = psp.tile([128, 512], F32, name=f"vps{i}", tag="qk", bufs=2)
            for ci in range(NC_):
                nc.tensor.matmul(
                    ps[:], xts[ci][:, 128 * m:128 * m + 128], wvs[ci][:],
                    start=(ci == 0), stop=(ci == NC_ - 1),
                )
            dst = vp[i][:, 0:520].rearrange("p (h e) -> p h e", e=65)[:, :, 0:64]
            src = ps[:].rearrange("p (h e) -> p h e", e=64)
            nc.vector.tensor_copy(dst, src)
            ocol = vp[i][:, 0:520].rearrange("p (h e) -> p h e", e=65)[:, :, 64:65]
            nc.vector.tensor_copy(ocol, ones_b[:].unsqueeze(2))
            return
        wsrc, bias_t = (wqs, bqs) if kind == "q" else (wks, bks)
        ps = psp.tile([128, 512], F32, name=f"{kind}ps{j}_{m}", tag="qk", bufs=2)
        for ci in range(NC_):
            nc.tensor.matmul(
                ps[:], wsrc[ci][:, 128 * m:128 * m + 128], xts[ci][:],
                start=(ci == 0), stop=(ci == NC_ - 1),
            )
        if kind == "k":
            out_ap = kt[m][:, 512 * j:512 * j + 512]
        else:
            t_ = pool.tile([128, 512], F32R, name=f"qt{m}_{j}", tag="qtc", bufs=16)
            qtc[m][j] = t_
            out_ap = t_[:]
        nc.vector.tensor_scalar_add(out_ap, ps[:], bias_t[:, m:m + 1])

    def qkv_units(j):
        # q first (unblocks attention), then k, then v
        return ([lambda j=j, m=m: qkv_chain(j, "q", m) for m in range(4)]
                + [lambda j=j, m=m: qkv_chain(j, "k", m) for m in range(4)]
                + [lambda j=j, m=m: qkv_chain(j, "v", m) for m in range(4)])

    def emit_qkv(j):
        emit_qkv_dma(j)
        for f in qkv_units(j):
            f()

    # ---- attention ----
    # S^T pair-slot descriptors for chunk j: list of slots, each a list of
    # (i, col0, t0, w, mask_col). AV consumption: col = col0 + 128*u - t0.
    def s_slots(j):
        out = []
        for p in range(2 * j):  # full pairs
            out.append([(2 * p, 0, 0, 512, None), (2 * p + 1, 512, 0, 512, None)])
        # diagonal pair A: tiles 4j (full width) and 4j+1 (t >= 128)
        out.append([(4 * j, 0, 0, 512, 0), (4 * j + 1, 512, 128, 384, 512)])
        # diagonal pair B: tiles 4j+2 and 4j+3, both 256 wide at t0=256
        # (tile 4j+3 only needs t in [384,512) but a 256-wide matmul avoids
        # the <256 fp32r 4x penalty; its cols [256,384) are acausal garbage
        # that the exp covers harmlessly and AV never reads)
        out.append([(4 * j + 2, 0, 256, 256, 0), (4 * j + 3, 256, 256, 256, 384)])
        return out

    av_pend = []  # cross-call head pipeline: AV trails S by one head
    tp_pend = []  # transposes trail their pair's AV by one head

    def s_units(j, h):
        mt, off = h // 2, 64 * (h % 2)
        ets = {}  # s-tile i -> (et tile, col0, t0)

        def emit_slot(slot):
            qsrc = qtc[mt][j]
            wtot = max(c0 + w for (_, c0, _, w, _) in slot)
            sp = psp.tile([128, 1024], F32, name=f"sp{h}_{j}", tag="sp", bufs=2)
            et = pool.tile([128, 1024], BF16, name=f"et{h}_{j}", tag="et",
                           bufs=ET_BUFS)
            for (i, c0, t0, w, _) in slot:
                nc.tensor.matmul(
                    sp[:, c0:c0 + w], kt[mt][off:off + 64, 128 * i:128 * i + 128],
                    qsrc[off:off + 64, t0:t0 + w], start=True, stop=True,
                )
            nc.scalar.activation(et[:, 0:wtot], sp[:, 0:wtot], AF.Exp, scale=0.125)
            for (i, c0, t0, w, mcol) in slot:
                if mcol is not None:
                    nc.vector.tensor_mul(et[:, mcol:mcol + 128],
                                         et[:, mcol:mcol + 128], tri[:])
                ets[i] = (et, c0, t0)

        units = [lambda slot=slot: emit_slot(slot) for slot in s_slots(j)]
        return units, ets

    def av_units(j, h, ets):
        mt, off = h // 2, 64 * (h % 2)
        op = psp.tile([128, 260], F32, name=f"o{h}_{j}", tag="o", bufs=2)

        def chain(u):
            oc = 65 * u
            lo = 4 * j + u + 1
            for i in range(lo):
                et, c0, t0 = ets[i]
                col = c0 + 128 * u - t0
                nc.tensor.matmul(
                    op[:, oc:oc + 65], et[:, col:col + 128],
                    vp[i][:, 65 * h:65 * h + 65],
                    start=(i == 0), stop=(i == lo - 1),
                )
            # normalize in [t, d]: Z is column 64 -> per-partition scalar
            rb = pool.tile([128, 1], F32, name=f"rb{h}_{j}_{u}", tag="rb", bufs=8)
            nc.vector.reciprocal(rb[:], op[:, oc + 64:oc + 65])
            if h % 2 == 0:
                nrmt[mt][u] = pool.tile([128, 128], F32R, name=f"nrm{mt}_{j}_{u}",
                                        tag="nrm", bufs=8)
            nc.vector.tensor_scalar_mul(
                nrmt[mt][u][:, off:off + 64], op[:, oc:oc + 64], rb[:]
            )

        def tp_unit(mt=mt, j=j, pair_nrm=nrmt[mt]):
            # pair complete: transpose [t, d-pair] -> [d-pair, t] for the proj
            tp = psp.tile([128, 512], F32R, name=f"tp{mt}_{j}", tag="qk", bufs=2)
            ot = pool.tile([128, 512], F32R, name=f"ot{mt}_{j}", tag="otc", bufs=16)
            otc[mt][j] = ot
            for u in range(4):
                nc.tensor.matmul(
                    tp[:, 128 * u:128 * u + 128], pair_nrm[u][:], ident[:],
                    is_transpose=True,
                )
                nc.vector.tensor_copy(
                    ot[:, 128 * u:128 * u + 128], tp[:, 128 * u:128 * u + 128]
                )

        units = [lambda u=u: chain(u) for u in range(4)]
        return units, (tp_unit if h % 2 == 1 else None)

    def attn_heads(j, heads, extras=()):
        """Emit S for each head, interleaving the previous head's AV chains,
        the previous pair's transposes and any extra PE work units
        (proj/QKV chains) between S pair-slots."""
        extras = list(extras)
        for h in heads:
            su, ets = s_units(j, h)
            # transposes delayed from an earlier head run first; the one
            # produced by this head's av_units must wait until the next head
            tpu_now = tp_pend.pop(0) if tp_pend else None
            au = []
            if av_pend:
                au, tpu = av_units(*av_pend.pop(0))
                if tpu is not None:
                    tp_pend.append(tpu)
            seq = [su[0]]
            if len(su) > 1:
                seq.append(su[1])
            if tpu_now is not None:
                seq.append(tpu_now)
            si, ai = 2, 0
            while si < len(su) or ai < len(au):
                if ai < len(au):
                    seq.append(au[ai])
                    ai += 1
                if si < len(su):
                    seq.append(su[si])
                    si += 1
            for f in seq:
                f()
            if extras:
                extras.pop(0)()
            av_pend.append((j, h, ets))
        for f in extras:
            f()

    def flush_av():
        while av_pend:
            au, tpu = av_units(*av_pend.pop(0))
            for f in au:
                f()
            if tpu is not None:
                tp_pend.append(tpu)
        while tp_pend:
            tp_pend.pop(0)()

    def emit_wp_loads():
        for m in range(4):
            for n in range(2):
                t_ = pool.tile([128, 512], F32R, name=f"wps{m}_{n}", tag="wp", bufs=8)
                wps[m][n] = t_
                nc.sync.dma_start(
                    t_[:],
                    wp[128 * m:128 * m + 128, 512 * n:512 * n + 512].bitcast(F32R),
                )

    def proj_chain(j, u, n):
        t = 4 * j + u
        ps = psp.tile([128, 512], F32, name=f"yps{t}_{n}", tag="qk", bufs=2)
        for m in range(4):
            nc.tensor.matmul(
                ps[:], otc[m][j][:, 128 * u:128 * u + 128], wps[m][n][:],
                start=(m == 0), stop=(m == 3),
            )
        yo = pool.tile([128, 512], BF16, name=f"yo{t}_{n}", tag="yo", bufs=4)
        nc.vector.tensor_copy(yo[:], ps[:])
        eng = nc.sync if (t + n) % 2 == 0 else nc.scalar
        eng.dma_start(
            yout[128 * t:128 * t + 128, 512 * n:512 * n + 512].bitcast(BF16), yo[:]
        )

    def proj_units(j):
        return [lambda j=j, u=u, n=n: proj_chain(j, u, n)
                for u in range(4) for n in range(2)]

    def group(units, sizes):
        out, k = [], 0
        for s in sizes:
            chunk = units[k:k + s]
            out.append(lambda chunk=chunk: [f() for f in chunk])
            k += s
        assert k == len(units)
        return out

    # ---- macro schedule ----
    # attention chunks in order 1,2,3,0: later chunks get QKV/proj chains as
    # PE filler against their exp-bound phases; the cheap chunk-0 exps drain
    # the pipeline, and proj(0) ends the program as pure PE+DMA work.
    emit_qkv(0)
    emit_qkv(1)
    emit_qkv_dma(2)
    attn_heads(1, range(8), extras=group(qkv_units(2), (2, 2, 2, 2, 1, 1, 1, 1)))
    emit_qkv_dma(3)
    attn_heads(2, range(8), extras=group(qkv_units(3), (2, 2, 2, 2, 1, 1, 1, 1)))
    emit_wp_loads()
    attn_heads(3, range(8),
               extras=group(proj_units(1) + proj_units(2),
                            (2, 2, 2, 2, 2, 2, 2, 2)))
    attn_heads(0, range(8), extras=group(proj_units(3), (0, 2, 2, 2, 2, 0, 0, 0)))
    flush_av()
    for f in proj_units(0):
        f()

    for m in range(4):
        qtc[m] = [None] * NJ
        otc[m] = [None] * NJ
    pool.release()
    psp.release()


def build(passes=1):
    key = ("nc", passes)
    if key in _CACHE:
        return _CACHE[key]
    nc = bacc.Bacc("TRN2", target_bir_lowering=False, debug=False,
                   num_devices=N_CORES)
    aps = {
        "xT": nc.dram_tensor("xT", [C, T], U16, kind="ExternalInput").ap(),
        "wq": nc.dram_tensor("wq", [C, CH], U16, kind="ExternalInput").ap(),
        "wk": nc.dram_tensor("wk", [C, CH], U16, kind="ExternalInput").ap(),
        "wv": nc.dram_tensor("wv", [C, CH], U16, kind="ExternalInput").ap(),
        "wp": nc.dram_tensor("wp", [CH, C], F32, kind="ExternalInput").ap(),
        "bq2": nc.dram_tensor("bq2", [128, 4], F32, kind="ExternalInput").ap(),
        "bk2": nc.dram_tensor("bk2", [128, 4], F32, kind="ExternalInput").ap(),
        "mask": nc.dram_tensor("mask", [128, 128], F32, kind="ExternalInput").ap(),
        "y": nc.dram_tensor("y", [T, C], U16, kind="ExternalOutput").ap(),
    }
    with tile.TileContext(nc) as tc:
        for _ in range(passes):
            _emit(nc, tc, aps)
    nc.compile()
    _CACHE[key] = nc
    return nc


def _bf16_bits(a):
    """float32 ndarray -> bfloat16 bit pattern as uint16 (round to nearest even)."""
    u = np.ascontiguousarray(a, dtype=np.float32).view(np.uint32)
    r = (u + 0x7FFF + ((u >> 16) & 1)) >> 16
    return r.astype(np.uint16)


def _bf16_to_f32(bits):
    return (bits.astype(np.uint32) << 16).view(np.float32)


def make_in_maps(x, Wq, bq, Wk, bk, Wv, bv, Wp, bp):
    # lower-triangle 0/1 mask for the diagonal 128x128 attention blocks
    s_idx = np.arange(128)[:, None]
    t_idx = np.arange(128)[None, :]
    mask = (s_idx <= t_idx).astype(np.float32)
    in_maps = []
    for c in range(N_CORES):
        b, g = c // 2, c % 2
        cols = slice(CH * g, CH * g + CH)
        in_maps.append({
            "xT": _bf16_bits(x[b].T),
            "wq": _bf16_bits(Wq[:, cols]),
            "wk": _bf16_bits(Wk[:, cols]),
            "wv": _bf16_bits(Wv[:, cols]),
            "wp": np.ascontiguousarray(Wp[cols, :]),
            "bq2": np.ascontiguousarray(bq[cols].reshape(4, 128).T),
            "bk2": np.ascontiguousarray(bk[cols].reshape(4, 128).T),
            "mask": mask,
        })
    return in_maps


def kernel(x, Wq, bq, Wk, bk, Wv, bv, Wp, bp):
    # host-side prep is pure numpy; convert in case jax arrays are passed
    x, Wq, bq, Wk, bk, Wv, bv, Wp, bp = (
        np.asarray(a, dtype=np.float32)
        for a in (x, Wq, bq, Wk, bk, Wv, bv, Wp, bp)
    )
    nc = build()
    in_maps = make_in_maps(x, Wq, bq, Wk, bk, Wv, bv, Wp, bp)
    # the axon-proxied device occasionally reports a transient unrecoverable
    # exec state that clears on a fresh attempt; retry rather than fail
    last_err = None
    for _attempt in range(3):
        try:
            res = run_bass_kernel_spmd(nc, in_maps, core_ids=list(range(N_CORES)))
            break
        except Exception as e:  # noqa: BLE001
            last_err = e
            import time as _time
            _time.sleep(5)
    else:
        raise last_err
    corr = (bv @ Wp + bp).astype(np.float32)
    out = np.empty((B, T, C), dtype=np.float32)
    for b in range(B):
        out[b] = (_bf16_to_f32(res.results[2 * b]["y"])
                  + _bf16_to_f32(res.results[2 * b + 1]["y"]) + corr)
    return out


# revision 9
# speedup vs baseline: 1.1382x; 1.0183x over previous
"""Causal self-attention (B=4, T=2048, C=1024, H=16, D=64) on 8 trn2 cores.

Sharding: data-parallel over B (4) x tensor-parallel over head-halves (2).
Core c handles batch c//2 with heads [8*(c%2), 8*(c%2)+8). Each core emits a
partial projection output [2048, 1024] (bf16); host sums the two head-half
partials per batch and adds the (bv @ Wp + bp) correction row.

Device layout highlights:
 - matmul cost on PE is proportional to the output free-dim size only, so the
   attention*V product is computed in [t, d] layout: per (head, t-block of
   128, s-tile) one bf16 matmul with a 65-wide output (64 V columns + a ones
   column that yields the softmax denominator Z per t-partition). That makes
   softmax normalization a per-partition reciprocal+tensor_scalar (no
   broadcast matmuls) and halves the AV stream cost vs. the [d, t] form.
 - normalized [t, d-pair] tiles are transposed back to [d-pair, t] on the PE
   (bf16 is_transpose matmul vs. a 128x128 identity) to feed the output
   projection; the transpose emission is delayed by one head so the DVE
   normalize chain never stalls the PE.
 - S^T = K^T.T @ Q^T stays fp32r in [s, t] layout in [128,1024] psum pair
   slots (one exp instruction per slot amortizes ACT's fixed per-instruction
   overhead); all S matmuls are >= 256 wide for the 1 cyc/row fp32r rate.
 - the attention phase is ACT(exp)-bound, so emission interleaves S pair
   units with the previous head's AV chains and with projection / QKV
   chains at fine granularity; attention chunks run in order 1,2,3,0 so the
   pipeline drains on the cheapest chunk's exps.
 - weights/x/y stream as bf16 in 1-2 large strided DMAs each: the HWDGE
   descriptor generator is a serial ~630ns/DMA resource, so few big
   transfers beat many tile-sized ones.
 - softmax skips max-subtraction (logits are ~N(0,1); exp cannot overflow)
 - causal masking via 0/1 mask multiply on the 4 diagonal-block patterns
"""

import os
import sys

for _p in ("/opt/trn_rl_repo", "/root/.axon_site/_ro/trn_rl_repo"):
    if os.path.isdir(_p) and _p not in sys.path:
        sys.path.insert(0, _p)

import numpy as np
from concourse import bacc, masks, mybir, tile
from concourse.bass_utils import run_bass_kernel_spmd

N_CORES = 8
B, T, C = 4, 2048, 1024
H, D = 16, 64          # full model heads
HG = 8                 # heads per core (head-group)
CH = HG * D            # 512, per-core qkv width
NT = T // 128          # 16 s-tiles
NJ = T // 512          # 4 t-chunks
NC_ = C // 128         # 8 contraction tiles
F32 = mybir.dt.float32
F32R = mybir.dt.float32r
BF16 = mybir.dt.bfloat16
U16 = mybir.dt.uint16
AF = mybir.ActivationFunctionType

ET_BUFS = 18           # [128,1024] bf16 S^T pair tiles across the head pipeline

_CACHE = {}


def _emit(nc, tc, aps):
    xT, wq, wk, wv, wp, bq2, bk2, mask, yout = (
        aps["xT"], aps["wq"], aps["wk"], aps["wv"], aps["wp"],
        aps["bq2"], aps["bk2"], aps["mask"], aps["y"],
    )

    pool = tc.alloc_tile_pool(name="pool", bufs=1)
    psp = tc.alloc_tile_pool(name="ps", bufs=1, space="PSUM")

    # ---- persistent tensors ----
    kt = [pool.tile([128, T], F32R, name=f"kt{m}", tag="kt", bufs=4) for m in range(4)]
    vp = [pool.tile([128, 520], BF16, name=f"vp{i}", tag="vp", bufs=NT)
          for i in range(NT)]
    # single lower-triangle mask (1{s <= t}) for the diagonal 128x128 blocks
    tri_f = pool.tile([128, 128], F32, name="tri_f", tag="tri_f", bufs=1)
    tri = pool.tile([128, 128], BF16, name="tri", tag="tri", bufs=1)
    ident = pool.tile([128, 128], BF16, name="ident", tag="ident", bufs=1)
    bqs = pool.tile([128, 4], F32, name="bqs", tag="bias", bufs=2)
    bks = pool.tile([128, 4], F32, name="bks", tag="bias", bufs=2)
    ones_b = pool.tile([128, 8], BF16, name="ones_b", tag="ones_b", bufs=1)

    # qkv weights: one [128, 8x512] tile per matrix, loaded in 1-2 big DMAs
    # (the HWDGE descriptor generator is serial at ~630ns/DMA)
    wqb = pool.tile([128, 4096], BF16, name="wqb", tag="wqkv", bufs=3)
    wkb = pool.tile([128, 4096], BF16, name="wkb", tag="wqkv", bufs=3)
    wvb = pool.tile([128, 4096], BF16, name="wvb", tag="wqkv", bufs=3)
    wqs = [wqb[:, 512 * ci:512 * ci + 512] for ci in range(NC_)]
    wks = [wkb[:, 512 * ci:512 * ci + 512] for ci in range(NC_)]
    wvs = [wvb[:, 512 * ci:512 * ci + 512] for ci in range(NC_)]
    wpb = pool.tile([128, 4096], BF16, name="wpb", tag="wp", bufs=1)
    wps = [[wpb[:, 1024 * m + 512 * n:1024 * m + 512 * n + 512] for n in range(2)]
           for m in range(4)]

    def _w3d(ap):  # [128, 4096] tile -> [128, 8, 512] view
        return ap.rearrange("p (ci c) -> p ci c", c=512)

    # startup DMAs: sync carries wq (split for earlier first tiles) then wk;
    # scalar carries biases, the first x chunk and wv in parallel.
    nc.sync.dma_start(_w3d(wqb[:])[:, 0:4],
                      wq[0:512, :].rearrange("(ci p) c -> p ci c", p=128).bitcast(BF16))
    nc.sync.dma_start(_w3d(wqb[:])[:, 4:8],
                      wq[512:1024, :].rearrange("(ci p) c -> p ci c", p=128).bitcast(BF16))
    nc.scalar.dma_start(bqs[:], bq2[:])
    nc.scalar.dma_start(bks[:], bk2[:])
    xtb = [pool.tile([128, 4096], BF16, name=f"xtb{j}", tag="xt", bufs=2)
           if j < 99 else None for j in range(NJ)]
    xts_all = [[xtb[j][:, 512 * ci:512 * ci + 512] for ci in range(NC_)]
               for j in range(NJ)]

    def _x3d(j, lo, hi):
        return (xtb[j][:].rearrange("p (ci c) -> p ci c", c=512)[:, lo:hi],
                xT[128 * lo:128 * hi, 512 * j:512 * j + 512]
                .rearrange("(ci p) c -> p ci c", p=128).bitcast(BF16))

    nc.scalar.dma_start(*_x3d(0, 0, 4))
    nc.scalar.dma_start(*_x3d(0, 4, 8))
    nc.sync.dma_start(_w3d(wkb[:]),
                      wk[:, :].rearrange("(ci p) c -> p ci c", p=128).bitcast(BF16))
    nc.scalar.dma_start(_w3d(wvb[:]),
                        wv[:, :].rearrange("(ci p) c -> p ci c", p=128).bitcast(BF16))
    nc.scalar.dma_start(tri_f[:], mask[:])
    nc.vector.tensor_copy(tri[:], tri_f[:])
    nc.gpsimd.memset(ones_b[:], 1.0)
    masks.make_identity(nc, ident[:])

    qtc = [[None] * NJ for _ in range(4)]   # per-chunk Q^T tiles
    otc = [[None] * NJ for _ in range(4)]   # per-chunk O^T tiles
    nrmt = [[None] * 4 for _ in range(4)]   # per-pair normalized [t, d-pair]

    def emit_qkv_dma(j):
        if j == 0:
            return
        nc.sync.dma_start(xtb[j][:].rearrange("p (ci c) -> p ci c", c=512),
                          xT[:, 512 * j:512 * j + 512]
                          .rearrange("(ci p) c -> p ci c", p=128).bitcast(BF16))

    def qkv_chain(j, kind, m):
        # one [128,512] psum accumulation chain of the q/k/v projections
        xts = xts_all[j]
        if kind == "v":
            i = 4 * j + m
            ps = psp.tile([128, 512], F32, name=f"vps{i}", tag="qk", bufs=2)
            for ci in range(NC_):
                nc.tensor.matmul(
                    ps[:], xts[ci][:, 128 * m:128 * m + 128], wvs[ci],
                    start=(ci == 0), stop=(ci == NC_ - 1),
                )
            dst = vp[i][:, 0:520].rearrange("p (h e) -> p h e", e=65)[:, :, 0:64]
            src = ps[:].rearrange("p (h e) -> p h e", e=64)
            nc.vector.tensor_copy(dst, src)
            ocol = vp[i][:, 0:520].rearrange("p (h e) -> p h e", e=65)[:, :, 64:65]
            nc.vector.tensor_copy(ocol, ones_b[:].unsqueeze(2))
            return
        wsrc, bias_t = (wqs, bqs) if kind == "q" else (wks, bks)
        ps = psp.tile([128, 512], F32, name=f"{kind}ps{j}_{m}", tag="qk", bufs=2)
        for ci in range(NC_):
            nc.tensor.matmul(
                ps[:], wsrc[ci][:, 128 * m:128 * m + 128], xts[ci][:],
                start=(ci == 0), stop=(ci == NC_ - 1),
            )
        if kind == "k":
            out_ap = kt[m][:, 512 * j:512 * j + 512]
        else:
            t_ = pool.tile([128, 512], F32R, name=f"qt{m}_{j}", tag="qtc", bufs=16)
            qtc[m][j] = t_
            out_ap = t_[:]
        nc.vector.tensor_scalar_add(out_ap, ps[:], bias_t[:, m:m + 1])

    def qkv_units(j):
        # q first (unblocks attention), then k, then v
        return ([lambda j=j, m=m: qkv_chain(j, "q", m) for m in range(4)]
                + [lambda j=j, m=m: qkv_chain(j, "k", m) for m in range(4)]
                + [lambda j=j, m=m: qkv_chain(j, "v", m) for m in range(4)])

    def emit_qkv(j):
        emit_qkv_dma(j)
        for f in qkv_units(j):
            f()

    # ---- attention ----
    # S^T pair-slot descriptors for chunk j: list of slots, each a list of
    # (i, col0, t0, w, mask_col). AV consumption: col = col0 + 128*u - t0.
    def s_slots(j):
        out = []
        for p in range(2 * j):  # full pairs
            out.append([(2 * p, 0, 0, 512, None), (2 * p + 1, 512, 0, 512, None)])
        # diagonal pair A: tiles 4j (full width) and 4j+1 (t >= 128)
        out.append([(4 * j, 0, 0, 512, 0), (4 * j + 1, 512, 128, 384, 512)])
        # diagonal pair B: tiles 4j+2 and 4j+3, both 256 wide at t0=256
        # (tile 4j+3 only needs t in [384,512) but a 256-wide matmul avoids
        # the <256 fp32r 4x penalty; its cols [256,384) are acausal garbage
        # that the exp covers harmlessly and AV never reads)
        out.append([(4 * j + 2, 0, 256, 256, 0), (4 * j + 3, 256, 256, 256, 384)])
        return out

    av_pend = []  # cross-call head pipeline: AV trails S by one head
    tp_pend = []  # transposes trail their pair's AV by one head

    def s_units(j, h):
        mt, off = h // 2, 64 * (h % 2)
        ets = {}  # s-tile i -> (et tile, col0, t0)

        def emit_slot(slot):
            qsrc = qtc[mt][j]
            wtot = max(c0 + w for (_, c0, _, w, _) in slot)
            sp = psp.tile([128, 1024], F32, name=f"sp{h}_{j}", tag="sp", bufs=2)
            et = pool.tile([128, 1024], BF16, name=f"et{h}_{j}", tag="et",
                           bufs=ET_BUFS)
            for (i, c0, t0, w, _) in slot:
                nc.tensor.matmul(
                    sp[:, c0:c0 + w], kt[mt][off:off + 64, 128 * i:128 * i + 128],
                    qsrc[off:off + 64, t0:t0 + w], start=True, stop=True,
                )
            nc.scalar.activation(et[:, 0:wtot], sp[:, 0:wtot], AF.Exp, scale=0.125)
            for (i, c0, t0, w, mcol) in slot:
                if mcol is not None:
                    nc.vector.tensor_mul(et[:, mcol:mcol + 128],
                                         et[:, mcol:mcol + 128], tri[:])
                ets[i] = (et, c0, t0)

        units = [lambda slot=slot: emit_slot(slot) for slot in s_slots(j)]
        return units, ets

    def av_units(j, h, ets):
        mt, off = h // 2, 64 * (h % 2)
        op = psp.tile([128, 260], F32, name=f"o{h}_{j}", tag="o", bufs=2)

        def chain(u):
            oc = 65 * u
            lo = 4 * j + u + 1
            for i in range(lo):
                et, c0, t0 = ets[i]
                col = c0 + 128 * u - t0
                nc.tensor.matmul(
                    op[:, oc:oc + 65], et[:, col:col + 128],
                    vp[i][:, 65 * h:65 * h + 65],
                    start=(i == 0), stop=(i == lo - 1),
                )
            # normalize in [t, d]: Z is column 64 -> per-partition scalar
            rb = pool.tile([128, 1], F32, name=f"rb{h}_{j}_{u}", tag="rb", bufs=8)
            nc.vector.reciprocal(rb[:], op[:, oc + 64:oc + 65])
            if h % 2 == 0:
                nrmt[mt][u] = pool.tile([128, 128], BF16, name=f"nrm{mt}_{j}_{u}",
                                        tag="nrm", bufs=8)
            nc.vector.tensor_scalar_mul(
                nrmt[mt][u][:, off:off + 64], op[:, oc:oc + 64], rb[:]
            )

        def tp_unit(mt=mt, j=j, pair_nrm=nrmt[mt]):
            # pair complete: transpose [t, d-pair] -> [d-pair, t] for the proj
            tp = psp.tile([128, 512], BF16, name=f"tp{mt}_{j}", tag="o", bufs=2)
            ot = pool.tile([128, 512], BF16, name=f"ot{mt}_{j}", tag="otc", bufs=16)
            otc[mt][j] = ot
            for u in range(4):
                nc.tensor.matmul(
                    tp[:, 128 * u:128 * u + 128], pair_nrm[u][:], ident[:],
                    is_transpose=True,
                )
                nc.vector.tensor_copy(
                    ot[:, 128 * u:128 * u + 128], tp[:, 128 * u:128 * u + 128]
                )

        units = [lambda u=u: chain(u) for u in range(4)]
        return units, (tp_unit if h % 2 == 1 else None)

    def attn_heads(j, heads, extras=()):
        """Emit S for each head, interleaving the previous head's AV chains,
        the previous pair's transposes and any extra PE work units
        (proj/QKV chains) between S pair-slots."""
        extras = list(extras)
        for h in heads:
            su, ets = s_units(j, h)
            # transposes delayed from an earlier head run first; the one
            # produced by this head's av_units must wait until the next head
            tpu_now = tp_pend.pop(0) if tp_pend else None
            au = []
            if av_pend:
                au, tpu = av_units(*av_pend.pop(0))
                if tpu is not None:
                    tp_pend.append(tpu)
            seq = [su[0]]
            if len(su) > 1:
                seq.append(su[1])
            if tpu_now is not None:
                seq.append(tpu_now)
            si, ai = 2, 0
            while si < len(su) or ai < len(au):
                if ai < len(au):
                    seq.append(au[ai])
                    ai += 1
                if si < len(su):
                    seq.append(su[si])
                    si += 1
            for f in seq:
                f()
            if extras:
                extras.pop(0)()
            av_pend.append((j, h, ets))
        for f in extras:
            f()

    def flush_av():
        while av_pend:
            au, tpu = av_units(*av_pend.pop(0))
            for f in au:
                f()
            if tpu is not None:
                tp_pend.append(tpu)
        while tp_pend:
            tp_pend.pop(0)()

    def emit_wp_loads():
        nc.sync.dma_start(wpb[:].rearrange("p (m c) -> p m c", c=1024),
                          wp[:, :].rearrange("(m p) c -> p m c", p=128).bitcast(BF16))

    def proj_unit(j, u):
        # both column halves of one [128 t, 1024] output row block + one DMA
        t = 4 * j + u
        yo = pool.tile([128, 1024], BF16, name=f"yo{t}", tag="yo", bufs=4)
        for n in range(2):
            ps = psp.tile([128, 512], F32, name=f"yps{t}_{n}", tag="qk", bufs=2)
            for m in range(4):
                nc.tensor.matmul(
                    ps[:], otc[m][j][:, 128 * u:128 * u + 128], wps[m][n],
                    start=(m == 0), stop=(m == 3),
                )
            nc.vector.tensor_copy(yo[:, 512 * n:512 * n + 512], ps[:])
        nc.sync.dma_start(
            yout[128 * t:128 * t + 128, :].bitcast(BF16), yo[:]
        )

    def proj_units(j):
        return [lambda j=j, u=u: proj_unit(j, u) for u in range(4)]

    def group(units, sizes):
        out, k = [], 0
        for s in sizes:
            chunk = units[k:k + s]
            out.append(lambda chunk=chunk: [f() for f in chunk])
            k += s
        assert k == len(units)
        return out

    # ---- macro schedule ----
    # attention chunks in order 1,2,3,0: later chunks get QKV/proj chains as
    # PE filler against their exp-bound phases; the cheap chunk-0 exps drain
    # the pipeline, and proj(0) ends the program as pure PE+DMA work.
    emit_qkv(0)
    emit_qkv(1)
    emit_qkv_dma(2)
    attn_heads(1, range(8), extras=group(qkv_units(2), (2, 2, 2, 2, 1, 1, 1, 1)))
    emit_qkv_dma(3)
    attn_heads(2, range(8), extras=group(qkv_units(3), (2, 2, 2, 2, 1, 1, 1, 1)))
    emit_wp_loads()
    attn_heads(3, range(8),
               extras=group(proj_units(1) + proj_units(2), (1,) * 8))
    attn_heads(0, range(8), extras=group(proj_units(3), (0, 1, 1, 0, 1, 0, 1, 0)))
    flush_av()
    for f in proj_units(0):
        f()

    for m in range(4):
        qtc[m] = [None] * NJ
        otc[m] = [None] * NJ
    pool.release()
    psp.release()


def build(passes=1):
    key = ("nc", passes)
    if key in _CACHE:
        return _CACHE[key]
    nc = bacc.Bacc("TRN2", target_bir_lowering=False, debug=False,
                   num_devices=N_CORES)
    aps = {
        "xT": nc.dram_tensor("xT", [C, T], U16, kind="ExternalInput").ap(),
        "wq": nc.dram_tensor("wq", [C, CH], U16, kind="ExternalInput").ap(),
        "wk": nc.dram_tensor("wk", [C, CH], U16, kind="ExternalInput").ap(),
        "wv": nc.dram_tensor("wv", [C, CH], U16, kind="ExternalInput").ap(),
        "wp": nc.dram_tensor("wp", [CH, C], U16, kind="ExternalInput").ap(),
        "bq2": nc.dram_tensor("bq2", [128, 4], F32, kind="ExternalInput").ap(),
        "bk2": nc.dram_tensor("bk2", [128, 4], F32, kind="ExternalInput").ap(),
        "mask": nc.dram_tensor("mask", [128, 128], F32, kind="ExternalInput").ap(),
        "y": nc.dram_tensor("y", [T, C], U16, kind="ExternalOutput").ap(),
    }
    with tile.TileContext(nc) as tc:
        for _ in range(passes):
            _emit(nc, tc, aps)
    nc.compile()
    _CACHE[key] = nc
    return nc


def _bf16_bits(a):
    """float32 ndarray -> bfloat16 bit pattern as uint16 (round to nearest even)."""
    u = np.ascontiguousarray(a, dtype=np.float32).view(np.uint32)
    r = (u + 0x7FFF + ((u >> 16) & 1)) >> 16
    return r.astype(np.uint16)


def _bf16_to_f32(bits):
    return (bits.astype(np.uint32) << 16).view(np.float32)


def make_in_maps(x, Wq, bq, Wk, bk, Wv, bv, Wp, bp):
    # lower-triangle 0/1 mask for the diagonal 128x128 attention blocks
    s_idx = np.arange(128)[:, None]
    t_idx = np.arange(128)[None, :]
    mask = (s_idx <= t_idx).astype(np.float32)
    in_maps = []
    for c in range(N_CORES):
        b, g = c // 2, c % 2
        cols = slice(CH * g, CH * g + CH)
        in_maps.append({
            "xT": _bf16_bits(x[b].T),
            "wq": _bf16_bits(Wq[:, cols]),
            "wk": _bf16_bits(Wk[:, cols]),
            "wv": _bf16_bits(Wv[:, cols]),
            "wp": _bf16_bits(Wp[cols, :]),
            "bq2": np.ascontiguousarray(bq[cols].reshape(4, 128).T),
            "bk2": np.ascontiguousarray(bk[cols].reshape(4, 128).T),
            "mask": mask,
        })
    return in_maps


def kernel(x, Wq, bq, Wk, bk, Wv, bv, Wp, bp):
    # host-side prep is pure numpy; convert in case jax arrays are passed
    x, Wq, bq, Wk, bk, Wv, bv, Wp, bp = (
        np.asarray(a, dtype=np.float32)
        for a in (x, Wq, bq, Wk, bk, Wv, bv, Wp, bp)
    )
    nc = build()
    in_maps = make_in_maps(x, Wq, bq, Wk, bk, Wv, bv, Wp, bp)
    # the axon-proxied device occasionally reports a transient unrecoverable
    # exec state that clears on a fresh attempt; retry rather than fail
    last_err = None
    for _attempt in range(3):
        try:
            res = run_bass_kernel_spmd(nc, in_maps, core_ids=list(range(N_CORES)))
            break
        except Exception as e:  # noqa: BLE001
            last_err = e
            import time as _time
            _time.sleep(5)
    else:
        raise last_err
    corr = (bv @ Wp + bp).astype(np.float32)
    out = np.empty((B, T, C), dtype=np.float32)
    for b in range(B):
        out[b] = (_bf16_to_f32(res.results[2 * b]["y"])
                  + _bf16_to_f32(res.results[2 * b + 1]["y"]) + corr)
    return out


# revision 11
# speedup vs baseline: 1.1759x; 1.0331x over previous
"""Causal self-attention (B=4, T=2048, C=1024, H=16, D=64) on 8 trn2 cores.

Sharding: data-parallel over B (4) x tensor-parallel over head-halves (2).
Core c handles batch c//2 with heads [8*(c%2), 8*(c%2)+8). Each core emits a
partial projection output [2048, 1024] (bf16); host sums the two head-half
partials per batch and adds the (bv @ Wp + bp) correction row.

Device layout highlights:
 - matmul cost on PE is proportional to the output free-dim size only, so the
   attention*V product is computed in [t, d] layout: per (head, t-block of
   128, s-tile) one bf16 matmul with a 65-wide output (64 V columns + a ones
   column that yields the softmax denominator Z per t-partition). That makes
   softmax normalization a per-partition reciprocal+tensor_scalar (no
   broadcast matmuls) and halves the AV stream cost vs. the [d, t] form.
 - normalized [t, d-pair] tiles are transposed back to [d-pair, t] on the PE
   (bf16 is_transpose matmul vs. a 128x128 identity) to feed the output
   projection; the transpose emission is delayed by one head so the DVE
   normalize chain never stalls the PE.
 - S^T = K^T.T @ Q^T stays fp32r in [s, t] layout in [128,1024] psum pair
   slots (one exp instruction per slot amortizes ACT's fixed per-instruction
   overhead); all S matmuls are >= 256 wide for the 1 cyc/row fp32r rate.
 - the attention phase is ACT(exp)-bound, so emission interleaves S pair
   units with the previous head's AV chains and with projection / QKV
   chains at fine granularity; attention chunks run in order 1,2,3,0 so the
   pipeline drains on the cheapest chunk's exps.
 - weights/x/y stream as bf16 in 1-2 large strided DMAs each: the HWDGE
   descriptor generator is a serial ~630ns/DMA resource, so few big
   transfers beat many tile-sized ones.
 - softmax skips max-subtraction (logits are ~N(0,1); exp cannot overflow)
 - causal masking via 0/1 mask multiply on the 4 diagonal-block patterns
"""

import os
import sys

for _p in ("/opt/trn_rl_repo", "/root/.axon_site/_ro/trn_rl_repo"):
    if os.path.isdir(_p) and _p not in sys.path:
        sys.path.insert(0, _p)

import numpy as np
from concourse import bacc, masks, mybir, tile
from concourse.bass_utils import run_bass_kernel_spmd

N_CORES = 8
B, T, C = 4, 2048, 1024
H, D = 16, 64          # full model heads
HG = 8                 # heads per core (head-group)
CH = HG * D            # 512, per-core qkv width
NT = T // 128          # 16 s-tiles
NJ = T // 512          # 4 t-chunks
NC_ = C // 128         # 8 contraction tiles
F32 = mybir.dt.float32
F32R = mybir.dt.float32r
BF16 = mybir.dt.bfloat16
U16 = mybir.dt.uint16
AF = mybir.ActivationFunctionType

ET_BUFS = 18           # [128,1024] bf16 S^T pair tiles across the head pipeline

_CACHE = {}


def _emit(nc, tc, aps):
    xT, wq, wk, wv, wp, bq2, bk2, mask, yout = (
        aps["xT"], aps["wq"], aps["wk"], aps["wv"], aps["wp"],
        aps["bq2"], aps["bk2"], aps["mask"], aps["y"],
    )

    pool = tc.alloc_tile_pool(name="pool", bufs=1)
    psp = tc.alloc_tile_pool(name="ps", bufs=1, space="PSUM")

    # ---- persistent tensors ----
    kt = [pool.tile([128, T], F32R, name=f"kt{m}", tag="kt", bufs=4) for m in range(4)]
    vp = [pool.tile([128, 520], BF16, name=f"vp{i}", tag="vp", bufs=NT)
          for i in range(NT)]
    # single lower-triangle mask (1{s <= t}) for the diagonal 128x128 blocks
    tri_f = pool.tile([128, 128], F32, name="tri_f", tag="tri_f", bufs=1)
    tri = pool.tile([128, 128], BF16, name="tri", tag="tri", bufs=1)
    ident = pool.tile([128, 128], BF16, name="ident", tag="ident", bufs=1)
    bqs = pool.tile([128, 4], F32, name="bqs", tag="bias", bufs=2)
    bks = pool.tile([128, 4], F32, name="bks", tag="bias", bufs=2)
    ones_b = pool.tile([128, 8], BF16, name="ones_b", tag="ones_b", bufs=1)

    # qkv weights: one [128, 8x512] tile per matrix, loaded in 1-2 big DMAs
    # (the HWDGE descriptor generator is serial at ~630ns/DMA)
    wqb = pool.tile([128, 4096], BF16, name="wqb", tag="wqkv", bufs=3)
    wkb = pool.tile([128, 4096], BF16, name="wkb", tag="wqkv", bufs=3)
    wvb = pool.tile([128, 4096], BF16, name="wvb", tag="wqkv", bufs=3)
    wqs = [wqb[:, 512 * ci:512 * ci + 512] for ci in range(NC_)]
    wks = [wkb[:, 512 * ci:512 * ci + 512] for ci in range(NC_)]
    wvs = [wvb[:, 512 * ci:512 * ci + 512] for ci in range(NC_)]
    wpb = pool.tile([128, 4096], BF16, name="wpb", tag="wp", bufs=1)
    wps = [[wpb[:, 1024 * m + 512 * n:1024 * m + 512 * n + 512] for n in range(2)]
           for m in range(4)]

    def _w3d(ap):  # [128, 4096] tile -> [128, 8, 512] view
        return ap.rearrange("p (ci c) -> p ci c", c=512)

    # startup DMAs: every large transfer serializes through the shared HWDGE
    # generator + DMA engines, so they all go on the sync queue in strict
    # consumption order (q operands, then k, v, next x chunk); the scalar
    # queue only carries the tiny bias/mask loads.
    xtb = [pool.tile([128, 4096], BF16, name=f"xtb{j}", tag="xt", bufs=2)
           for j in range(NJ)]
    xts_all = [[xtb[j][:, 512 * ci:512 * ci + 512] for ci in range(NC_)]
               for j in range(NJ)]

    def _x3d(j, lo, hi):
        return (xtb[j][:].rearrange("p (ci c) -> p ci c", c=512)[:, lo:hi],
                xT[128 * lo:128 * hi, 512 * j:512 * j + 512]
                .rearrange("(ci p) c -> p ci c", p=128).bitcast(BF16))

    nc.sync.dma_start(_w3d(wqb[:])[:, 0:4],
                      wq[0:512, :].rearrange("(ci p) c -> p ci c", p=128).bitcast(BF16))
    nc.scalar.dma_start(bqs[:], bq2[:])
    nc.scalar.dma_start(bks[:], bk2[:])
    nc.sync.dma_start(*_x3d(0, 0, 4))
    nc.sync.dma_start(_w3d(wqb[:])[:, 4:8],
                      wq[512:1024, :].rearrange("(ci p) c -> p ci c", p=128).bitcast(BF16))
    nc.sync.dma_start(*_x3d(0, 4, 8))
    nc.sync.dma_start(_w3d(wkb[:]),
                      wk[:, :].rearrange("(ci p) c -> p ci c", p=128).bitcast(BF16))
    nc.sync.dma_start(_w3d(wvb[:]),
                      wv[:, :].rearrange("(ci p) c -> p ci c", p=128).bitcast(BF16))
    nc.scalar.dma_start(tri_f[:], mask[:])
    nc.vector.tensor_copy(tri[:], tri_f[:])
    nc.gpsimd.memset(ones_b[:], 1.0)
    masks.make_identity(nc, ident[:])

    qtc = [[None] * NJ for _ in range(4)]   # per-chunk Q^T tiles
    otc = [[None] * NJ for _ in range(4)]   # per-chunk O^T tiles
    nrmt = [[None] * 4 for _ in range(4)]   # per-pair normalized [t, d-pair]

    def emit_qkv_dma(j):
        if j == 0:
            return
        nc.sync.dma_start(xtb[j][:].rearrange("p (ci c) -> p ci c", c=512),
                          xT[:, 512 * j:512 * j + 512]
                          .rearrange("(ci p) c -> p ci c", p=128).bitcast(BF16))

    def qkv_chain(j, kind, m):
        # one [128,512] psum accumulation chain of the q/k/v projections
        xts = xts_all[j]
        if kind == "v":
            i = 4 * j + m
            ps = psp.tile([128, 512], F32, name=f"vps{i}", tag="qk", bufs=2)
            for ci in range(NC_):
                nc.tensor.matmul(
                    ps[:], xts[ci][:, 128 * m:128 * m + 128], wvs[ci],
                    start=(ci == 0), stop=(ci == NC_ - 1),
                )
            dst = vp[i][:, 0:520].rearrange("p (h e) -> p h e", e=65)[:, :, 0:64]
            src = ps[:].rearrange("p (h e) -> p h e", e=64)
            nc.vector.tensor_copy(dst, src)
            ocol = vp[i][:, 0:520].rearrange("p (h e) -> p h e", e=65)[:, :, 64:65]
            nc.vector.tensor_copy(ocol, ones_b[:].unsqueeze(2))
            return
        wsrc, bias_t = (wqs, bqs) if kind == "q" else (wks, bks)
        ps = psp.tile([128, 512], F32, name=f"{kind}ps{j}_{m}", tag="qk", bufs=2)
        for ci in range(NC_):
            nc.tensor.matmul(
                ps[:], wsrc[ci][:, 128 * m:128 * m + 128], xts[ci][:],
                start=(ci == 0), stop=(ci == NC_ - 1),
            )
        if kind == "k":
            out_ap = kt[m][:, 512 * j:512 * j + 512]
        else:
            t_ = pool.tile([128, 512], F32R, name=f"qt{m}_{j}", tag="qtc", bufs=16)
            qtc[m][j] = t_
            out_ap = t_[:]
        nc.vector.tensor_scalar_add(out_ap, ps[:], bias_t[:, m:m + 1])

    def qkv_units(j):
        # q first (unblocks attention), then k, then v
        return ([lambda j=j, m=m: qkv_chain(j, "q", m) for m in range(4)]
                + [lambda j=j, m=m: qkv_chain(j, "k", m) for m in range(4)]
                + [lambda j=j, m=m: qkv_chain(j, "v", m) for m in range(4)])

    def emit_qkv(j):
        emit_qkv_dma(j)
        for f in qkv_units(j):
            f()

    # ---- attention ----
    # S^T pair-slot descriptors for chunk j: list of slots, each a list of
    # (i, col0, t0, w, mask_col). AV consumption: col = col0 + 128*u - t0.
    def s_slots(j):
        out = []
        for p in range(2 * j):  # full pairs
            out.append([(2 * p, 0, 0, 512, None), (2 * p + 1, 512, 0, 512, None)])
        # diagonal pair A: tiles 4j (full width) and 4j+1 (t >= 128)
        out.append([(4 * j, 0, 0, 512, 0), (4 * j + 1, 512, 128, 384, 512)])
        # diagonal pair B: tiles 4j+2 and 4j+3, both 256 wide at t0=256
        # (tile 4j+3 only needs t in [384,512) but a 256-wide matmul avoids
        # the <256 fp32r 4x penalty; its cols [256,384) are acausal garbage
        # that the exp covers harmlessly and AV never reads)
        out.append([(4 * j + 2, 0, 256, 256, 0), (4 * j + 3, 256, 256, 256, 384)])
        return out

    av_pend = []  # cross-call head pipeline: AV trails S by one head
    tp_pend = []  # transposes trail their pair's AV by one head

    def s_units(j, h):
        mt, off = h // 2, 64 * (h % 2)
        ets = {}  # s-tile i -> (et tile, col0, t0)

        def emit_slot(slot):
            qsrc = qtc[mt][j]
            wtot = max(c0 + w for (_, c0, _, w, _) in slot)
            sp = psp.tile([128, 1024], F32, name=f"sp{h}_{j}", tag="sp", bufs=2)
            et = pool.tile([128, 1024], BF16, name=f"et{h}_{j}", tag="et",
                           bufs=ET_BUFS)
            for (i, c0, t0, w, _) in slot:
                nc.tensor.matmul(
                    sp[:, c0:c0 + w], kt[mt][off:off + 64, 128 * i:128 * i + 128],
                    qsrc[off:off + 64, t0:t0 + w], start=True, stop=True,
                )
            nc.scalar.activation(et[:, 0:wtot], sp[:, 0:wtot], AF.Exp, scale=0.125)
            for (i, c0, t0, w, mcol) in slot:
                if mcol is not None:
                    nc.vector.tensor_mul(et[:, mcol:mcol + 128],
                                         et[:, mcol:mcol + 128], tri[:])
                ets[i] = (et, c0, t0)

        units = [lambda slot=slot: emit_slot(slot) for slot in s_slots(j)]
        return units, ets

    def av_units(j, h, ets):
        mt, off = h // 2, 64 * (h % 2)
        op = psp.tile([128, 260], F32, name=f"o{h}_{j}", tag="o", bufs=2)

        def chain(u):
            oc = 65 * u
            lo = 4 * j + u + 1
            for i in range(lo):
                et, c0, t0 = ets[i]
                col = c0 + 128 * u - t0
                nc.tensor.matmul(
                    op[:, oc:oc + 65], et[:, col:col + 128],
                    vp[i][:, 65 * h:65 * h + 65],
                    start=(i == 0), stop=(i == lo - 1),
                )
            # normalize in [t, d]: Z is column 64 -> per-partition scalar
            rb = pool.tile([128, 1], F32, name=f"rb{h}_{j}_{u}", tag="rb", bufs=8)
            nc.vector.reciprocal(rb[:], op[:, oc + 64:oc + 65])
            if h % 2 == 0:
                nrmt[mt][u] = pool.tile([128, 128], BF16, name=f"nrm{mt}_{j}_{u}",
                                        tag="nrm", bufs=8)
            nc.vector.tensor_scalar_mul(
                nrmt[mt][u][:, off:off + 64], op[:, oc:oc + 64], rb[:]
            )

        def tp_unit(mt=mt, j=j, pair_nrm=nrmt[mt]):
            # pair complete: transpose [t, d-pair] -> [d-pair, t] for the proj
            tp = psp.tile([128, 512], BF16, name=f"tp{mt}_{j}", tag="o", bufs=2)
            ot = pool.tile([128, 512], BF16, name=f"ot{mt}_{j}", tag="otc", bufs=16)
            otc[mt][j] = ot
            for u in range(4):
                nc.tensor.matmul(
                    tp[:, 128 * u:128 * u + 128], pair_nrm[u][:], ident[:],
                    is_transpose=True,
                )
                nc.vector.tensor_copy(
                    ot[:, 128 * u:128 * u + 128], tp[:, 128 * u:128 * u + 128]
                )

        units = [lambda u=u: chain(u) for u in range(4)]
        return units, (tp_unit if h % 2 == 1 else None)

    def attn_heads(j, heads, extras=()):
        """Emit S for each head, interleaving the previous head's AV chains,
        the previous pair's transposes and any extra PE work units
        (proj/QKV chains) between S pair-slots."""
        extras = list(extras)
        for h in heads:
            su, ets = s_units(j, h)
            # transposes delayed from an earlier head run first; the one
            # produced by this head's av_units must wait until the next head
            tpu_now = tp_pend.pop(0) if tp_pend else None
            au = []
            if av_pend:
                au, tpu = av_units(*av_pend.pop(0))
                if tpu is not None:
                    tp_pend.append(tpu)
            seq = [su[0]]
            if len(su) > 1:
                seq.append(su[1])
            if tpu_now is not None:
                seq.append(tpu_now)
            si, ai = 2, 0
            while si < len(su) or ai < len(au):
                if ai < len(au):
                    seq.append(au[ai])
                    ai += 1
                if si < len(su):
                    seq.append(su[si])
                    si += 1
            for f in seq:
                f()
            if extras:
                extras.pop(0)()
            av_pend.append((j, h, ets))
        for f in extras:
            f()

    def flush_av():
        while av_pend:
            au, tpu = av_units(*av_pend.pop(0))
            for f in au:
                f()
            if tpu is not None:
                tp_pend.append(tpu)
        while tp_pend:
            tp_pend.pop(0)()

    def emit_wp_loads():
        nc.sync.dma_start(wpb[:].rearrange("p (m c) -> p m c", c=1024),
                          wp[:, :].rearrange("(m p) c -> p m c", p=128).bitcast(BF16))

    def proj_unit(j, u):
        # both column halves of one [128 t, 1024] output row block + one DMA
        t = 4 * j + u
        yo = pool.tile([128, 1024], BF16, name=f"yo{t}", tag="yo", bufs=4)
        for n in range(2):
            ps = psp.tile([128, 512], F32, name=f"yps{t}_{n}", tag="qk", bufs=2)
            for m in range(4):
                nc.tensor.matmul(
                    ps[:], otc[m][j][:, 128 * u:128 * u + 128], wps[m][n],
                    start=(m == 0), stop=(m == 3),
                )
            nc.vector.tensor_copy(yo[:, 512 * n:512 * n + 512], ps[:])
        nc.sync.dma_start(
            yout[128 * t:128 * t + 128, :].bitcast(BF16), yo[:]
        )

    def proj_units(j):
        return [lambda j=j, u=u: proj_unit(j, u) for u in range(4)]

    def group(units, sizes):
        out, k = [], 0
        for s in sizes:
            chunk = units[k:k + s]
            out.append(lambda chunk=chunk: [f() for f in chunk])
            k += s
        assert k == len(units)
        return out

    # ---- macro schedule ----
    # attention chunks in order 1,2,3,0: later chunks get QKV/proj chains as
    # PE filler against their exp-bound phases; the cheap chunk-0 exps drain
    # the pipeline, and proj(0) ends the program as pure PE+DMA work.
    emit_qkv(0)
    emit_qkv(1)
    emit_qkv_dma(2)
    attn_heads(1, range(8), extras=group(qkv_units(2), (2, 2, 2, 2, 1, 1, 1, 1)))
    emit_qkv_dma(3)
    attn_heads(2, range(8), extras=group(qkv_units(3), (2, 2, 2, 2, 1, 1, 1, 1)))
    emit_wp_loads()
    attn_heads(3, range(8),
               extras=group(proj_units(1) + proj_units(2), (1,) * 8))
    attn_heads(0, range(8), extras=group(proj_units(3), (0, 0, 0, 0, 1, 1, 1, 1)))
    flush_av()
    for f in proj_units(0):
        f()

    for m in range(4):
        qtc[m] = [None] * NJ
        otc[m] = [None] * NJ
    pool.release()
    psp.release()


def build(passes=1):
    key = ("nc", passes)
    if key in _CACHE:
        return _CACHE[key]
    nc = bacc.Bacc("TRN2", target_bir_lowering=False, debug=False,
                   num_devices=N_CORES)
    aps = {
        "xT": nc.dram_tensor("xT", [C, T], U16, kind="ExternalInput").ap(),
        "wq": nc.dram_tensor("wq", [C, CH], U16, kind="ExternalInput").ap(),
        "wk": nc.dram_tensor("wk", [C, CH], U16, kind="ExternalInput").ap(),
        "wv": nc.dram_tensor("wv", [C, CH], U16, kind="ExternalInput").ap(),
        "wp": nc.dram_tensor("wp", [CH, C], U16, kind="ExternalInput").ap(),
        "bq2": nc.dram_tensor("bq2", [128, 4], F32, kind="ExternalInput").ap(),
        "bk2": nc.dram_tensor("bk2", [128, 4], F32, kind="ExternalInput").ap(),
        "mask": nc.dram_tensor("mask", [128, 128], F32, kind="ExternalInput").ap(),
        "y": nc.dram_tensor("y", [T, C], U16, kind="ExternalOutput").ap(),
    }
    with tile.TileContext(nc) as tc:
        for _ in range(passes):
            _emit(nc, tc, aps)
    nc.compile()
    _CACHE[key] = nc
    return nc


def _bf16_bits(a):
    """float32 ndarray -> bfloat16 bit pattern as uint16 (round to nearest even)."""
    u = np.ascontiguousarray(a, dtype=np.float32).view(np.uint32)
    r = (u + 0x7FFF + ((u >> 16) & 1)) >> 16
    return r.astype(np.uint16)


def _bf16_to_f32(bits):
    return (bits.astype(np.uint32) << 16).view(np.float32)


def make_in_maps(x, Wq, bq, Wk, bk, Wv, bv, Wp, bp):
    # lower-triangle 0/1 mask for the diagonal 128x128 attention blocks
    s_idx = np.arange(128)[:, None]
    t_idx = np.arange(128)[None, :]
    mask = (s_idx <= t_idx).astype(np.float32)
    in_maps = []
    for c in range(N_CORES):
        b, g = c // 2, c % 2
        cols = slice(CH * g, CH * g + CH)
        in_maps.append({
            "xT": _bf16_bits(x[b].T),
            "wq": _bf16_bits(Wq[:, cols]),
            "wk": _bf16_bits(Wk[:, cols]),
            "wv": _bf16_bits(Wv[:, cols]),
            "wp": _bf16_bits(Wp[cols, :]),
            "bq2": np.ascontiguousarray(bq[cols].reshape(4, 128).T),
            "bk2": np.ascontiguousarray(bk[cols].reshape(4, 128).T),
            "mask": mask,
        })
    return in_maps


def kernel(x, Wq, bq, Wk, bk, Wv, bv, Wp, bp):
    # host-side prep is pure numpy; convert in case jax arrays are passed
    x, Wq, bq, Wk, bk, Wv, bv, Wp, bp = (
        np.asarray(a, dtype=np.float32)
        for a in (x, Wq, bq, Wk, bk, Wv, bv, Wp, bp)
    )
    nc = build()
    in_maps = make_in_maps(x, Wq, bq, Wk, bk, Wv, bv, Wp, bp)
    # the axon-proxied device occasionally reports a transient unrecoverable
    # exec state that clears on a fresh attempt; retry rather than fail
    last_err = None
    for _attempt in range(3):
        try:
            res = run_bass_kernel_spmd(nc, in_maps, core_ids=list(range(N_CORES)))
            break
        except Exception as e:  # noqa: BLE001
            last_err = e
            import time as _time
            _time.sleep(5)
    else:
        raise last_err
    corr = (bv @ Wp + bp).astype(np.float32)
    out = np.empty((B, T, C), dtype=np.float32)
    for b in range(B):
        out[b] = (_bf16_to_f32(res.results[2 * b]["y"])
                  + _bf16_to_f32(res.results[2 * b + 1]["y"]) + corr)
    return out
